# revision 1
# baseline (speedup 1.0000x reference)
"""Trainium2 Bass kernel for a single-head attention block (B=4, S=2048, D=1024).

reference:
    x = gelu(tokens); q,k,v = x@W{q,k,v} + b; scores = q@k^T/sqrt(D)
    out = softmax(scores)@v @ Wo + bo + tokens

Sharding: 8 cores = 4 batches x 2 query-halves. Core c=2b+h handles batch b and
query rows [h*1024, (h+1)*1024). Each core computes q/k/v projections for its
own rows only; K^T and V halves are exchanged pairwise via two AllGathers (key/
value order is the natural batch order on every core). The fp32 residual path
dominates the output magnitude, so the whole matmul pipeline runs in fp8-e4m3
with DoubleRow perf mode (K=256 per matmul, ~1.44x bf16 throughput).

Scales: weights are pre-scaled x32 on the host (sigma~1 in fp8), so stored
q,k,v are 32x true scale. scores_psum = 32768*scores_true -> exp uses
scale=2^-15, bias=-5ln2, giving expT = exp(scores)/32 in fp8. Softmax
denominators via a ones-stationary matmul (S_psum = Sigma exp/32); rS_row =
1/(32*S_psum) = 1/Sigma exp is broadcast across partitions via a DRAM
round-trip (stride-0 partition DMA), and the mixed psum (Sigma exp*v, which
can reach ~7000 because gelu output has a +0.4 mean -> v columns have a DC
component) is normalized to the true `mixed` on the psum->fp8 DVE convert.
The out-proj psum is then 32*(mixed@Wo), folded by a constant 1/32.

Per-core device pipeline:
  warm-up MMs                          # keep PE HAM at K=8/8 through the head
  xTq = gelu(tokTq)                    # own rows, [d, sq], fp8
  kTo[e,sq] = (Wk^T-stat) xTq          # -> AllGather 1 -> kT[e,s]
  vo[sq,d]  = (xTq-stat) Wv            # -> AllGather 2 -> v[s,d]
  qT[e,sq]  = (Wq^T-stat) xTq
  scoresT[sk,sq] = kT qT               # DoubleRow, K=256 tiles
  expT = exp(scoresT/2^15 - 5ln2)      # ACT, no max-subtraction (|s|<3)
  S[sq] = ones^T expT                  # 1-col-stationary matmuls on PE
  mixedUT[d,sq] = (v-stat) expT / S    # normalized on the psum->fp8 convert
  out[sq,e] = mixedUT^T Wo / 32 + (residual + bo)
"""

import math

import numpy as np
import ml_dtypes

B, S, D = 4, 2048, 1024
NCORES = 8
SQ = S // 2          # query rows per core
P = 128
DT = 8               # d / 128
KP = DT // 2         # K-pair count for DoubleRow (K=256 each)
ST = S // P          # 16 seq tiles
SQT = SQ // P        # 8
N512 = 512
WARMUP_MMS = 48
WSCALE = 32.0        # host-side weight/bias scale
EXP_BIAS = -5.0 * math.log(2.0)   # expT = exp(scores)/32
EXP_SCALE = 1.0 / 32768.0         # scores_psum = 32768 * scores_true
# rS_row = 1/S_psum = 32/Sigma exp, so mixUT = 32*(mixed - c) in fp8 (the
# host centers v by c ~ E[v] so this sits in e4m3's precision sweet spot);
# out-proj psum = 32*(mixUT) @ (32*Wo) -> descale by 1/1024.
OUT_DESCALE = 1.0 / 1024.0
GELU_MEAN = 0.3989422804014327    # E[gelu(z)], z ~ N(0,1)

_COMPILED = {}


def _build_program():
    from contextlib import ExitStack

    import concourse.bass as bass
    import concourse.tile as tile
    from concourse import bacc, mybir

    f32 = mybir.dt.float32
    bf16 = mybir.dt.bfloat16
    f8 = mybir.dt.float8e4
    AF = mybir.ActivationFunctionType
    DR = mybir.MatmulPerfMode.DoubleRow

    nc = bacc.Bacc("TRN2", target_bir_lowering=False, debug=False,
                   num_devices=NCORES)

    tokTq = nc.dram_tensor("tokTq", [D, SQ], bf16, kind="ExternalInput")
    resid = nc.dram_tensor("resid", [SQ, D], f32, kind="ExternalInput")
    wq = nc.dram_tensor("wq", [D, D], f8, kind="ExternalInput")
    wk = nc.dram_tensor("wk", [D, D], f8, kind="ExternalInput")
    wv = nc.dram_tensor("wv", [D, D], f8, kind="ExternalInput")
    wo = nc.dram_tensor("wo", [D, D], f8, kind="ExternalInput")
    bq_d = nc.dram_tensor("bq", [D], f32, kind="ExternalInput")   # x32
    bk_d = nc.dram_tensor("bk", [D], f32, kind="ExternalInput")   # x32
    bv_d = nc.dram_tensor("bv", [D], f32, kind="ExternalInput")   # x32
    bo_d = nc.dram_tensor("bo", [D], f32, kind="ExternalInput")   # true scale
    out_d = nc.dram_tensor("out", [SQ, D], f32, kind="ExternalOutput")

    ts = bass.ts
    groups = [[2 * i, 2 * i + 1] for i in range(NCORES // 2)]

    with tile.TileContext(nc) as tc, ExitStack() as ctx:
        pers = ctx.enter_context(tc.tile_pool(name="pers", bufs=1))
        kT = pers.tile([P, DT, S], f8, tag="kT")
        qT = pers.tile([P, DT, SQ], f8, tag="qT")
        v = pers.tile([P, ST, D], f8, tag="v")
        ones = pers.tile([P, 2, 16], f8, tag="ones")
        bqk = pers.tile([P, 2, DT], f32, tag="bqk")  # [:,0,:]=32bq [:,1,:]=32bk
        ebias = pers.tile([P, 1], f32, tag="ebias")
        wscr = pers.tile([P, N512], bf16, tag="wscr")
        wsink = pers.tile([P, P], f32, tag="wsink")

        dram = ctx.enter_context(tc.tile_pool(name="dram", bufs=1, space="DRAM"))
        kb_in = dram.tile([D, SQ], f8, tag="kb_in")
        kb_out = dram.tile([2, D, SQ], f8, tag="kb_out")
        vb_in = dram.tile([SQ, D], f8, tag="vb_in")
        vb_out = dram.tile([2, SQ, D], f8, tag="vb_out")

        psum = ctx.enter_context(tc.tile_pool(name="psum", bufs=4, space="PSUM"))
        psum_s = ctx.enter_context(tc.tile_pool(name="psum_s", bufs=2, space="PSUM"))

        # --- PE warm-up: dense trivial matmuls so HAM hits K=8/8 and PE is
        # busy while the gelu+DMA head runs.
        nc.vector.memset(wscr, 0.0)
        wps = psum.tile([P, N512], f32, tag="mm")
        for i in range(WARMUP_MMS):
            nc.tensor.matmul(wps, wscr[:, :P], wscr, start=(i == 0),
                             stop=(i == WARMUP_MMS - 1))
        nc.vector.tensor_copy(wsink, wps[:, :P])

        nc.vector.memset(ones, 1.0)
        nc.vector.memset(ebias, EXP_BIAS)
        nc.sync.dma_start(bqk[:, 0, :], bq_d.ap().rearrange("(t p) -> p t", p=P))
        nc.sync.dma_start(bqk[:, 1, :], bk_d.ap().rearrange("(t p) -> p t", p=P))

        # ---------------- phase 1: gelu + projections + kT/v exchange -------
        with ExitStack() as ph1:
            p1 = ph1.enter_context(tc.tile_pool(name="p1", bufs=1))
            xTq = p1.tile([P, DT, SQ], f8, tag="xTq")
            wk_sb = p1.tile([P, DT, D], f8, tag="wk")
            wq_sb = p1.tile([P, DT, D], f8, tag="wq")
            wv_sb = p1.tile([P, DT, D], f8, tag="wv")
            bv_sb = p1.tile([P, D], f32, tag="bv")
            kTo = p1.tile([P, DT, SQ], f8, tag="kTo")
            vo = p1.tile([P, SQT, D], f8, tag="vo")
            stag = ph1.enter_context(tc.tile_pool(name="stag", bufs=8))

            nc.gpsimd.dma_start(wk_sb,
                                wk.ap().rearrange("(t p) e -> p t e", p=P))
            for t in range(DT):
                stq = stag.tile([P, SQ], bf16, tag="tok", name=f"stq{t}")
                nc.sync.dma_start(stq, tokTq.ap()[ts(t, P), :])
                nc.scalar.activation(xTq[:, t, :], stq, AF.Gelu)
            nc.gpsimd.dma_start(wv_sb,
                                wv.ap().rearrange("(t p) e -> p t e", p=P))
            nc.gpsimd.dma_start(wq_sb,
                                wq.ap().rearrange("(t p) e -> p t e", p=P))
            nc.gpsimd.dma_start(
                bv_sb, bass.AP(tensor=bv_d, offset=0, ap=[[0, P], [1, D]]))

            # kTo: lhsT = Wk-slice, rhs = xTq  -> exchange ASAP. Stream each
            # te row-block to DRAM as its converts land so the AllGather
            # trigger only waits on the last small DMA.
            kb_in_v = kb_in[:].rearrange("(t p) s -> p t s", p=P)
            for te in range(DT):
                for c in range(SQ // N512):
                    ps = psum.tile([P, N512], f32, tag="mm")
                    for u in range(KP):
                        nc.tensor.matmul(ps, wk_sb[:, 2 * u:2 * u + 2, ts(te, P)],
                                         xTq[:, 2 * u:2 * u + 2, ts(c, N512)],
                                         start=(u == 0), stop=(u == KP - 1),
                                         perf_mode=DR)
                    nc.vector.tensor_scalar_add(kTo[:, te, ts(c, N512)], ps,
                                                bqk[:, 1, te:te + 1])
                nc.sync.dma_start(kb_in_v[:, te, :], kTo[:, te, :])
            nc.gpsimd.collective_compute(
                "AllGather", mybir.AluOpType.bypass, replica_groups=groups,
                ins=[kb_in[:].opt()], outs=[kb_out[:].opt()])
            for r in range(2):
                for hh in range(2):
                    nc.sync.dma_start(
                        kT[:, :, r * SQ + hh * N512:r * SQ + (hh + 1) * N512],
                        kb_out[r][:, hh * N512:(hh + 1) * N512].rearrange(
                            "(t p) s -> p t s", p=P))

            # vo : lhsT = xTq-slice, rhs = Wv (own rows)  -> exchange
            vb_in_v = vb_in[:].rearrange("(t p) d -> p t d", p=P)
            for tsq in range(SQT):
                for dc in range(D // N512):
                    ps = psum.tile([P, N512], f32, tag="mm")
                    for u in range(KP):
                        nc.tensor.matmul(ps, xTq[:, 2 * u:2 * u + 2, ts(tsq, P)],
                                         wv_sb[:, 2 * u:2 * u + 2, ts(dc, N512)],
                                         start=(u == 0), stop=(u == KP - 1),
                                         perf_mode=DR)
                    nc.vector.tensor_add(vo[:, tsq, ts(dc, N512)], ps,
                                         bv_sb[:, ts(dc, N512)])
                nc.sync.dma_start(vb_in_v[:, tsq, :], vo[:, tsq, :])
            nc.gpsimd.collective_compute(
                "AllGather", mybir.AluOpType.bypass, replica_groups=groups,
                ins=[vb_in[:].opt()], outs=[vb_out[:].opt()])
            for r in range(2):
                nc.sync.dma_start(
                    v[:, r * SQT:(r + 1) * SQT, :],
                    vb_out[r][:].rearrange("(t p) d -> p t d", p=P))

            # qT : lhsT = Wq-slice, rhs = xTq
            for te in range(DT):
                for c in range(SQ // N512):
                    ps = psum.tile([P, N512], f32, tag="mm")
                    for u in range(KP):
                        nc.tensor.matmul(ps, wq_sb[:, 2 * u:2 * u + 2, ts(te, P)],
                                         xTq[:, 2 * u:2 * u + 2, ts(c, N512)],
                                         start=(u == 0), stop=(u == KP - 1),
                                         perf_mode=DR)
                    nc.vector.tensor_scalar_add(qT[:, te, ts(c, N512)], ps,
                                                bqk[:, 0, te:te + 1])

        # ---------------- phase 2: attention + out-proj ----------------
        with ExitStack() as ph2:
            w2pool = ph2.enter_context(tc.tile_pool(name="w2", bufs=1))
            wo_sb = w2pool.tile([P, DT, D], f8, tag="wo")
            bo_sb = w2pool.tile([P, D], f32, tag="bo")
            nc.sync.dma_start(wo_sb, wo.ap().rearrange("(t p) e -> p t e", p=P))
            nc.gpsimd.dma_start(
                bo_sb, bass.AP(tensor=bo_d, offset=0, ap=[[0, P], [1, D]]))
            epool = ph2.enter_context(tc.tile_pool(name="ep", bufs=2))
            work = ph2.enter_context(tc.tile_pool(name="wk2", bufs=2))
            rpool = ph2.enter_context(tc.tile_pool(name="rp", bufs=8))
            dpool = ph2.enter_context(
                tc.tile_pool(name="dram2", bufs=2, space="DRAM"))

            for c in range(SQ // N512):          # sq chunks of 512
                rs_dram = dpool.tile([N512], f32, tag="rs_dram")
                expT = epool.tile([P, ST, N512], f8, tag="expT")
                for tk in range(ST):
                    ps = psum.tile([P, N512], f32, tag="mm")
                    for u in range(KP):
                        nc.tensor.matmul(ps, kT[:, 2 * u:2 * u + 2, ts(tk, P)],
                                         qT[:, 2 * u:2 * u + 2, ts(c, N512)],
                                         start=(u == 0), stop=(u == KP - 1),
                                         perf_mode=DR)
                    nc.scalar.activation(expT[:, tk, :], ps, AF.Exp,
                                         scale=EXP_SCALE, bias=ebias)

                # softmax denominators: ones as 1-col stationary -> S [1, sq]
                psS = psum_s.tile([1, N512], f32, tag="S")
                for tk in range(ST // 2):
                    nc.tensor.matmul(psS, ones[:, :, :1],
                                     expT[:, 2 * tk:2 * tk + 2, :],
                                     start=(tk == 0), stop=(tk == ST // 2 - 1),
                                     perf_mode=DR)
                rS_row = work.tile([1, N512], f32, tag="rS_row")
                nc.vector.reciprocal(rS_row, psS)   # = 32 / Sigma exp
                # broadcast 1/S across partitions via DRAM (stride-0 DMA;
                # direct SBUF->SBUF partition-scatter DMA mis-writes)
                nc.sync.dma_start(
                    rs_dram[:].rearrange("(o s) -> o s", o=1), rS_row)
                rSb = work.tile([P, N512], f32, tag="rSb")
                nc.gpsimd.dma_start(rSb, rs_dram[:].partition_broadcast(P))

                # prefetch this chunk's residual rows (+bo) during the mixUT
                # window: early enough to unblock the out-proj tail, late
                # enough not to steal SBUF write bandwidth from scores
                res_sbs = []
                for sl in range(4):
                    row = (c * 4 + sl) * P
                    res_sb = rpool.tile([P, D], f32, tag="res",
                                        name=f"res{c}_{sl}")
                    nc.sync.dma_start(res_sb, resid.ap()[row:row + P, :])
                    nc.gpsimd.tensor_add(res_sb, res_sb, bo_sb)
                    res_sbs.append(res_sb)

                # mixedUT[d, sq] = (v^T-stationary @ expT) / S  (normalized on
                # the psum->fp8 convert; unnormalized would overflow e4m3)
                mixUT = work.tile([P, DT, N512], f8, tag="mixUT")
                for dsl in range(DT):
                    ps = psum.tile([P, N512], f32, tag="mm")
                    for tk in range(ST // 2):
                        nc.tensor.matmul(ps, v[:, 2 * tk:2 * tk + 2, ts(dsl, P)],
                                         expT[:, 2 * tk:2 * tk + 2, :],
                                         start=(tk == 0), stop=(tk == ST // 2 - 1),
                                         perf_mode=DR)
                    nc.vector.tensor_mul(mixUT[:, dsl, :], ps, rSb)

                for sl in range(4):
                    row = (c * 4 + sl) * P
                    res_sb = res_sbs[sl]
                    out_sb = work.tile([P, D], f32, tag="osb")
                    for ec in range(D // N512):
                        ps = psum.tile([P, N512], f32, tag="mm")
                        for u in range(KP):
                            nc.tensor.matmul(
                                ps, mixUT[:, 2 * u:2 * u + 2, ts(sl, P)],
                                wo_sb[:, 2 * u:2 * u + 2, ts(ec, N512)],
                                start=(u == 0), stop=(u == KP - 1),
                                perf_mode=DR)
                        # out = psum / 32 + (residual + bo)
                        nc.vector.tensor_scalar_mul(out_sb[:, ts(ec, N512)], ps,
                                                    OUT_DESCALE)
                        nc.vector.tensor_add(out_sb[:, ts(ec, N512)],
                                             out_sb[:, ts(ec, N512)],
                                             res_sb[:, ts(ec, N512)])
                    nc.sync.dma_start(out_d.ap()[row:row + P, :], out_sb)

    nc.compile()
    return nc


def _get_program():
    if "nc" not in _COMPILED:
        _COMPILED["nc"] = _build_program()
    return _COMPILED["nc"]


def make_in_maps(tokens, Wq, bq, Wk, bk, Wv, bv, Wo, bo):
    tokens = np.asarray(tokens, dtype=np.float32)
    bf = ml_dtypes.bfloat16
    f8 = ml_dtypes.float8_e4m3
    wq_b = np.ascontiguousarray((np.asarray(Wq, np.float32) * WSCALE).astype(f8))
    wk_b = np.ascontiguousarray((np.asarray(Wk, np.float32) * WSCALE).astype(f8))
    wv_b = np.ascontiguousarray((np.asarray(Wv, np.float32) * WSCALE).astype(f8))
    wo_b = np.ascontiguousarray((np.asarray(Wo, np.float32) * WSCALE).astype(f8))
    bq = np.asarray(bq, np.float32) * WSCALE
    bk = np.asarray(bk, np.float32) * WSCALE
    # center v by c ~ E_k[v] so the fp8 mixUT quantizes the small AC part;
    # softmax weights sum to 1, so out = (mixed-c)@Wo + (c@Wo + bo) + resid.
    wv32 = np.asarray(Wv, np.float32)
    cvec = GELU_MEAN * wv32.sum(axis=0) + np.asarray(bv, np.float32)
    bv = (np.asarray(bv, np.float32) - cvec) * WSCALE
    bo = np.asarray(bo, np.float32) + cvec @ np.asarray(Wo, np.float32)

    in_maps = []
    for c in range(NCORES):
        b, h = divmod(c, 2)
        q_rows = tokens[b, h * SQ:(h + 1) * SQ]
        in_maps.append({
            "tokTq": np.ascontiguousarray(q_rows.T.astype(bf)),  # [D, SQ]
            "resid": np.ascontiguousarray(q_rows),               # [SQ, D] f32
            "wq": wq_b, "wk": wk_b, "wv": wv_b, "wo": wo_b,
            "bq": bq, "bk": bk, "bv": bv, "bo": bo,
        })
    return in_maps


def gather_out(results):
    out = np.empty((B, S, D), np.float32)
    for c in range(NCORES):
        b, h = divmod(c, 2)
        out[b, h * SQ:(h + 1) * SQ] = results[c]["out"]
    return out


def kernel(tokens, Wq, bq, Wk, bk, Wv, bv, Wo, bo):
    from concourse.bass_utils import run_bass_kernel_spmd

    in_maps = make_in_maps(tokens, Wq, bq, Wk, bk, Wv, bv, Wo, bo)
    nc = _get_program()
    res = run_bass_kernel_spmd(nc, in_maps, core_ids=list(range(NCORES)),
                               trace=False)
    return gather_out(res.results)



# revision 4
# speedup vs baseline: 1.0030x; 1.0030x over previous
"""Trainium2 Bass kernel for a single-head attention block (B=4, S=2048, D=1024).

reference:
    x = gelu(tokens); q,k,v = x@W{q,k,v} + b; scores = q@k^T/sqrt(D)
    out = softmax(scores)@v @ Wo + bo + tokens

Sharding: 8 cores = 4 batches x 2 query-halves. Core c=2b+h handles batch b and
query rows [h*1024, (h+1)*1024). Each core computes q/k/v projections for its
own rows only; K^T and V halves are exchanged pairwise via two AllGathers (key/
value order is the natural batch order on every core). The fp32 residual path
dominates the output magnitude, so the whole matmul pipeline runs in fp8-e4m3
with DoubleRow perf mode (K=256 per matmul, ~1.44x bf16 throughput).

Scales: weights are pre-scaled x32 on the host (sigma~1 in fp8), so stored
q,k,v are 32x true scale. scores_psum = 32768*scores_true -> exp uses
scale=2^-15, bias=-5ln2, giving expT = exp(scores)/32 in fp8. Softmax
denominators via a ones-stationary matmul (S_psum = Sigma exp/32); rS_row =
1/(32*S_psum) = 1/Sigma exp is broadcast across partitions via a DRAM
round-trip (stride-0 partition DMA), and the mixed psum (Sigma exp*v, which
can reach ~7000 because gelu output has a +0.4 mean -> v columns have a DC
component) is normalized to the true `mixed` on the psum->fp8 DVE convert.
The out-proj psum is then 32*(mixed@Wo), folded by a constant 1/1024 on the
fused (psum*c + residual) DVE op; bo and the v-centering correction are
pre-added into the residual on the host.

v2 schedule notes (vs v1):
  - PSUM evictions alternate ACT/DVE so the Vector engine no longer gates
    the projection pipeline (v1: all evicts on DVE lagged ~20us, delaying
    the AllGather staging and everything downstream).
  - Phase 2 runs scores(c0), S(c0), scores(c1), S(c1), mix(c0), out(c0),
    mix(c1), out(c1): each chunk's softmax-denominator reciprocal + DRAM
    partition-broadcast round-trip hides behind the other chunk's scores.
  - kT/v unpack DMAs moved to the gpsimd queue so their AllGather sem-waits
    cannot block the sync queue's staging DMAs.
  - out-proj evict is one fused scalar_tensor_tensor (psum/1024 + residual).
"""

import math

import numpy as np
import ml_dtypes

B, S, D = 4, 2048, 1024
NCORES = 8
SQ = S // 2          # query rows per core
P = 128
DT = 8               # d / 128
KP = DT // 2         # K-pair count for DoubleRow (K=256 each)
ST = S // P          # 16 seq tiles
SQT = SQ // P        # 8
N512 = 512
WARMUP_MMS = 36
WSCALE = 32.0        # host-side weight/bias scale
EXP_BIAS = -5.0 * math.log(2.0)   # expT = exp(scores)/32
EXP_SCALE = 1.0 / 32768.0         # scores_psum = 32768 * scores_true
OUT_DESCALE = 1.0 / 1024.0
GELU_MEAN = 0.3989422804014327    # E[gelu(z)], z ~ N(0,1)

_COMPILED = {}


def _build_program():
    from contextlib import ExitStack

    import concourse.bass as bass
    import concourse.tile as tile
    from concourse import bacc, mybir

    f32 = mybir.dt.float32
    bf16 = mybir.dt.bfloat16
    f8 = mybir.dt.float8e4
    AF = mybir.ActivationFunctionType
    ALU = mybir.AluOpType
    DR = mybir.MatmulPerfMode.DoubleRow

    nc = bacc.Bacc("TRN2", target_bir_lowering=False, debug=False,
                   num_devices=NCORES)

    tokTq = nc.dram_tensor("tokTq", [D, SQ], bf16, kind="ExternalInput")
    resid = nc.dram_tensor("resid", [SQ, D], f32, kind="ExternalInput")
    wq = nc.dram_tensor("wq", [D, D], f8, kind="ExternalInput")
    wk = nc.dram_tensor("wk", [D, D], f8, kind="ExternalInput")
    wv = nc.dram_tensor("wv", [D, D], f8, kind="ExternalInput")
    wo = nc.dram_tensor("wo", [D, D], f8, kind="ExternalInput")
    bq_d = nc.dram_tensor("bq", [D], f32, kind="ExternalInput")   # x32
    bk_d = nc.dram_tensor("bk", [D], f32, kind="ExternalInput")   # x32
    bv_d = nc.dram_tensor("bv", [D], f32, kind="ExternalInput")   # x32
    out_d = nc.dram_tensor("out", [SQ, D], f32, kind="ExternalOutput")

    ts = bass.ts
    groups = [[2 * i, 2 * i + 1] for i in range(NCORES // 2)]

    with tile.TileContext(nc) as tc, ExitStack() as ctx:
        pers = ctx.enter_context(tc.tile_pool(name="pers", bufs=1))
        kT = pers.tile([P, DT, S], f8, tag="kT")
        qT = pers.tile([P, DT, SQ], f8, tag="qT")
        v = pers.tile([P, ST, D], f8, tag="v")
        ones = pers.tile([P, 2, 16], f8, tag="ones")
        bqk = pers.tile([P, 2, DT], f32, tag="bqk")  # [:,0,:]=32bq [:,1,:]=32bk
        ebias = pers.tile([P, 1], f32, tag="ebias")
        wscr = pers.tile([P, N512], bf16, tag="wscr")
        wsink = pers.tile([P, P], f32, tag="wsink")
        wo_sb = pers.tile([P, DT, D], f8, tag="wo")

        dram = ctx.enter_context(tc.tile_pool(name="dram", bufs=1, space="DRAM"))
        kb_in = dram.tile([D, SQ], f8, tag="kb_in")
        kb_out = dram.tile([2, D, SQ], f8, tag="kb_out")
        vb_in = dram.tile([SQ, D], f8, tag="vb_in")
        vb_out = dram.tile([2, SQ, D], f8, tag="vb_out")

        psum = ctx.enter_context(tc.tile_pool(name="psum", bufs=5, space="PSUM"))
        psum_s = ctx.enter_context(tc.tile_pool(name="psum_s", bufs=2, space="PSUM"))

        # --- PE warm-up: dense trivial matmuls so HAM hits K=8/8 and PE is
        # busy while the gelu+DMA head runs.
        nc.vector.memset(wscr, 0.0)
        wps = psum.tile([P, N512], f32, tag="mm")
        for i in range(WARMUP_MMS):
            nc.tensor.matmul(wps, wscr[:, :P], wscr, start=(i == 0),
                             stop=(i == WARMUP_MMS - 1))
        nc.vector.tensor_copy(wsink, wps[:, :P])

        nc.vector.memset(ones, 1.0)
        nc.vector.memset(ebias, EXP_BIAS)
        nc.sync.dma_start(bqk[:, 0, :], bq_d.ap().rearrange("(t p) -> p t", p=P))
        nc.sync.dma_start(bqk[:, 1, :], bk_d.ap().rearrange("(t p) -> p t", p=P))

        # ---------------- phase 1: gelu + projections + kT/v exchange -------
        with ExitStack() as ph1:
            p1 = ph1.enter_context(tc.tile_pool(name="p1", bufs=1))
            xTq = p1.tile([P, DT, SQ], f8, tag="xTq")
            wk_sb = p1.tile([P, DT, D], f8, tag="wk")
            wq_sb = p1.tile([P, DT, D], f8, tag="wq")
            wv_sb = p1.tile([P, DT, D], f8, tag="wv")
            bv_sb = p1.tile([P, D], f32, tag="bv")
            kTo = p1.tile([P, DT, SQ], f8, tag="kTo")
            vo = p1.tile([P, SQT, D], f8, tag="vo")
            stag = ph1.enter_context(tc.tile_pool(name="stag", bufs=8))

            nc.gpsimd.dma_start(wk_sb,
                                wk.ap().rearrange("(t p) e -> p t e", p=P))
            for t in range(DT):
                stq = stag.tile([P, SQ], bf16, tag="tok", name=f"stq{t}")
                eng = nc.sync if (t % 2 == 0) else nc.scalar
                eng.dma_start(stq, tokTq.ap()[ts(t, P), :])
                nc.scalar.activation(xTq[:, t, :], stq, AF.Gelu)
            nc.gpsimd.dma_start(wv_sb,
                                wv.ap().rearrange("(t p) e -> p t e", p=P))
            nc.gpsimd.dma_start(wq_sb,
                                wq.ap().rearrange("(t p) e -> p t e", p=P))
            nc.gpsimd.dma_start(wo_sb,
                                wo.ap().rearrange("(t p) e -> p t e", p=P))
            nc.gpsimd.dma_start(
                bv_sb, bass.AP(tensor=bv_d, offset=0, ap=[[0, P], [1, D]]))

            # kTo: lhsT = Wk-slice, rhs = xTq  -> exchange ASAP. Stream each
            # te row-block to DRAM as its converts land so the AllGather
            # trigger only waits on the last small DMA. The two chunk evicts
            # of each te run on ACT and DVE in parallel.
            kb_in_v = kb_in[:].rearrange("(t p) s -> p t s", p=P)
            for te in range(DT):
                for c in range(SQ // N512):
                    ps = psum.tile([P, N512], f32, tag="mm")
                    for u in range(KP):
                        nc.tensor.matmul(ps, wk_sb[:, 2 * u:2 * u + 2, ts(te, P)],
                                         xTq[:, 2 * u:2 * u + 2, ts(c, N512)],
                                         start=(u == 0), stop=(u == KP - 1),
                                         perf_mode=DR)
                    if c == 0:
                        nc.scalar.activation(kTo[:, te, ts(c, N512)], ps,
                                             AF.Identity, bias=bqk[:, 1, te:te + 1])
                    else:
                        nc.vector.tensor_scalar_add(kTo[:, te, ts(c, N512)], ps,
                                                    bqk[:, 1, te:te + 1])
                nc.sync.dma_start(kb_in_v[:, te, :], kTo[:, te, :])
            nc.gpsimd.collective_compute(
                "AllGather", mybir.AluOpType.bypass, replica_groups=groups,
                ins=[kb_in[:].opt()], outs=[kb_out[:].opt()])
            for r in range(2):
                for hh in range(2):
                    nc.gpsimd.dma_start(
                        kT[:, :, r * SQ + hh * N512:r * SQ + (hh + 1) * N512],
                        kb_out[r][:, hh * N512:(hh + 1) * N512].rearrange(
                            "(t p) s -> p t s", p=P))

            # vo : lhsT = xTq-slice, rhs = Wv (own rows)  -> exchange
            vb_in_v = vb_in[:].rearrange("(t p) d -> p t d", p=P)
            for tsq in range(SQT):
                for dc in range(D // N512):
                    ps = psum.tile([P, N512], f32, tag="mm")
                    for u in range(KP):
                        nc.tensor.matmul(ps, xTq[:, 2 * u:2 * u + 2, ts(tsq, P)],
                                         wv_sb[:, 2 * u:2 * u + 2, ts(dc, N512)],
                                         start=(u == 0), stop=(u == KP - 1),
                                         perf_mode=DR)
                    nc.vector.tensor_add(vo[:, tsq, ts(dc, N512)], ps,
                                         bv_sb[:, ts(dc, N512)])
                nc.sync.dma_start(vb_in_v[:, tsq, :], vo[:, tsq, :])
            nc.gpsimd.collective_compute(
                "AllGather", mybir.AluOpType.bypass, replica_groups=groups,
                ins=[vb_in[:].opt()], outs=[vb_out[:].opt()])
            for r in range(2):
                nc.gpsimd.dma_start(
                    v[:, r * SQT:(r + 1) * SQT, :],
                    vb_out[r][:].rearrange("(t p) d -> p t d", p=P))

            # qT : lhsT = Wq-slice, rhs = xTq
            for te in range(DT):
                for c in range(SQ // N512):
                    ps = psum.tile([P, N512], f32, tag="mm")
                    for u in range(KP):
                        nc.tensor.matmul(ps, wq_sb[:, 2 * u:2 * u + 2, ts(te, P)],
                                         xTq[:, 2 * u:2 * u + 2, ts(c, N512)],
                                         start=(u == 0), stop=(u == KP - 1),
                                         perf_mode=DR)
                    if c == 0:
                        nc.scalar.activation(qT[:, te, ts(c, N512)], ps,
                                             AF.Identity, bias=bqk[:, 0, te:te + 1])
                    else:
                        nc.vector.tensor_scalar_add(qT[:, te, ts(c, N512)], ps,
                                                    bqk[:, 0, te:te + 1])

        # ---------------- phase 2: attention + out-proj ----------------
        with ExitStack() as ph2:
            epool = ph2.enter_context(tc.tile_pool(name="ep", bufs=2))
            work = ph2.enter_context(tc.tile_pool(name="wk2", bufs=2))
            opool = ph2.enter_context(tc.tile_pool(name="op2", bufs=2))
            rspool = ph2.enter_context(tc.tile_pool(name="rs2", bufs=2))
            rpool = ph2.enter_context(tc.tile_pool(name="rp", bufs=8))
            dpool = ph2.enter_context(
                tc.tile_pool(name="dram2", bufs=2, space="DRAM"))

            # prefetch all residual rows now (vector queue; consumed by the
            # fused out-proj evict)
            res_sbs = []
            for sl8 in range(SQT):
                res_sb = rpool.tile([P, D], f32, tag="res", name=f"res{sl8}")
                nc.scalar.dma_start(res_sb, resid.ap()[sl8 * P:(sl8 + 1) * P, :])
                res_sbs.append(res_sb)

            # scores + softmax denominators for BOTH chunks first: each
            # chunk's reciprocal + DRAM broadcast round-trip hides behind
            # the other chunk's score matmuls.
            expTs, rSbs = [], []
            for c in range(SQ // N512):          # sq chunks of 512
                expT = epool.tile([P, ST, N512], f8, tag="expT",
                                  name=f"expT{c}")
                for tk in range(ST):
                    ps = psum.tile([P, N512], f32, tag="mm")
                    for u in range(KP):
                        nc.tensor.matmul(ps, kT[:, 2 * u:2 * u + 2, ts(tk, P)],
                                         qT[:, 2 * u:2 * u + 2, ts(c, N512)],
                                         start=(u == 0), stop=(u == KP - 1),
                                         perf_mode=DR)
                    nc.scalar.activation(expT[:, tk, :], ps, AF.Exp,
                                         scale=EXP_SCALE, bias=ebias)

                # softmax denominators: ones as 1-col stationary -> S [1, sq]
                psS = psum_s.tile([1, N512], f32, tag="S")
                for tk in range(ST // 2):
                    nc.tensor.matmul(psS, ones[:, :, :1],
                                     expT[:, 2 * tk:2 * tk + 2, :],
                                     start=(tk == 0), stop=(tk == ST // 2 - 1),
                                     perf_mode=DR)
                rS_row = rspool.tile([1, N512], f32, tag="rS_row",
                                     name=f"rS{c}")
                nc.vector.reciprocal(rS_row, psS)   # = 32 / Sigma exp
                # broadcast 1/S across partitions via DRAM (stride-0 DMA;
                # direct SBUF->SBUF partition-scatter DMA mis-writes)
                rs_dram = dpool.tile([N512], f32, tag="rs_dram")
                nc.sync.dma_start(
                    rs_dram[:].rearrange("(o s) -> o s", o=1), rS_row)
                rSb = rspool.tile([P, N512], f32, tag="rSb", name=f"rSb{c}")
                nc.gpsimd.dma_start(rSb, rs_dram[:].partition_broadcast(P))
                expTs.append(expT)
                rSbs.append(rSb)

            for c in range(SQ // N512):
                expT, rSb = expTs[c], rSbs[c]
                # mixedUT[d, sq] = (v^T-stationary @ expT) / S  (normalized on
                # the psum->fp8 convert; unnormalized would overflow e4m3)
                mixUT = work.tile([P, DT, N512], f8, tag="mixUT",
                                  name=f"mixUT{c}")
                for dsl in range(DT):
                    ps = psum.tile([P, N512], f32, tag="mm")
                    for tk in range(ST // 2):
                        nc.tensor.matmul(ps, v[:, 2 * tk:2 * tk + 2, ts(dsl, P)],
                                         expT[:, 2 * tk:2 * tk + 2, :],
                                         start=(tk == 0), stop=(tk == ST // 2 - 1),
                                         perf_mode=DR)
                    nc.vector.tensor_mul(mixUT[:, dsl, :], ps, rSb)

                for sl in range(4):
                    row = (c * 4 + sl) * P
                    res_sb = res_sbs[c * 4 + sl]
                    out_sb = opool.tile([P, D], f32, tag="osb")
                    for ec in range(D // N512):
                        ps = psum.tile([P, N512], f32, tag="mm")
                        for u in range(KP):
                            nc.tensor.matmul(
                                ps, mixUT[:, 2 * u:2 * u + 2, ts(sl, P)],
                                wo_sb[:, 2 * u:2 * u + 2, ts(ec, N512)],
                                start=(u == 0), stop=(u == KP - 1),
                                perf_mode=DR)
                        # out = psum / 1024 + (residual + bo), one fused op
                        nc.vector.scalar_tensor_tensor(
                            out_sb[:, ts(ec, N512)], ps, OUT_DESCALE,
                            res_sb[:, ts(ec, N512)], ALU.mult, ALU.add)
                    nc.sync.dma_start(out_d.ap()[row:row + P, :], out_sb)

    nc.compile()
    return nc


def _get_program():
    if "nc" not in _COMPILED:
        _COMPILED["nc"] = _build_program()
    return _COMPILED["nc"]


def make_in_maps(tokens, Wq, bq, Wk, bk, Wv, bv, Wo, bo):
    tokens = np.asarray(tokens, dtype=np.float32)
    bf = ml_dtypes.bfloat16
    f8 = ml_dtypes.float8_e4m3
    wq_b = np.ascontiguousarray((np.asarray(Wq, np.float32) * WSCALE).astype(f8))
    wk_b = np.ascontiguousarray((np.asarray(Wk, np.float32) * WSCALE).astype(f8))
    wv_b = np.ascontiguousarray((np.asarray(Wv, np.float32) * WSCALE).astype(f8))
    wo_b = np.ascontiguousarray((np.asarray(Wo, np.float32) * WSCALE).astype(f8))
    bq = np.asarray(bq, np.float32) * WSCALE
    bk = np.asarray(bk, np.float32) * WSCALE
    # center v by c ~ E_k[v] so the fp8 mixUT quantizes the small AC part;
    # softmax weights sum to 1, so out = (mixed-c)@Wo + (c@Wo + bo) + resid.
    # bo and the centering correction are folded into the residual host-side.
    wv32 = np.asarray(Wv, np.float32)
    cvec = GELU_MEAN * wv32.sum(axis=0) + np.asarray(bv, np.float32)
    bv = (np.asarray(bv, np.float32) - cvec) * WSCALE
    bo_eff = (np.asarray(bo, np.float32)
              + cvec @ np.asarray(Wo, np.float32)).astype(np.float32)

    in_maps = []
    for c in range(NCORES):
        b, h = divmod(c, 2)
        q_rows = tokens[b, h * SQ:(h + 1) * SQ]
        in_maps.append({
            "tokTq": np.ascontiguousarray(q_rows.T.astype(bf)),  # [D, SQ]
            "resid": np.ascontiguousarray(q_rows + bo_eff),      # [SQ, D] f32
            "wq": wq_b, "wk": wk_b, "wv": wv_b, "wo": wo_b,
            "bq": bq, "bk": bk, "bv": bv,
        })
    return in_maps


def gather_out(results):
    out = np.empty((B, S, D), np.float32)
    for c in range(NCORES):
        b, h = divmod(c, 2)
        out[b, h * SQ:(h + 1) * SQ] = results[c]["out"]
    return out


def kernel(tokens, Wq, bq, Wk, bk, Wv, bv, Wo, bo):
    from concourse.bass_utils import run_bass_kernel_spmd

    in_maps = make_in_maps(tokens, Wq, bq, Wk, bk, Wv, bv, Wo, bo)
    nc = _get_program()
    res = run_bass_kernel_spmd(nc, in_maps, core_ids=list(range(NCORES)),
                               trace=False)
    return gather_out(res.results)


# revision 10
# speedup vs baseline: 1.0178x; 1.0147x over previous
"""Trainium2 Bass kernel for a single-head attention block (B=4, S=2048, D=1024).

reference:
    x = gelu(tokens); q,k,v = x@W{q,k,v} + b; scores = q@k^T/sqrt(D)
    out = softmax(scores)@v @ Wo + bo + tokens

Sharding: 8 cores = 4 batches x 2 query-halves. Core c=2b+h handles batch b and
query rows [h*1024, (h+1)*1024). Each core computes q/k/v projections for its
own rows only; K^T and V halves are exchanged pairwise via two AllGathers (key/
value order is the natural batch order on every core). The fp32 residual path
dominates the output magnitude, so the whole matmul pipeline runs in fp8-e4m3
with DoubleRow perf mode (K=256 per matmul, ~1.44x bf16 throughput).

Scales: weights are pre-scaled x32 on the host (sigma~1 in fp8), so stored
q,k,v are 32x true scale. scores_psum = 32768*scores_true -> exp uses
scale=2^-15, bias=-5ln2, giving expT = exp(scores)/32 in fp8. Softmax
denominators via a ones-stationary matmul (S_psum = Sigma exp/32); rS_row =
1/(32*S_psum) = 1/Sigma exp is broadcast across partitions via a DRAM
round-trip (stride-0 partition DMA), and the mixed psum (Sigma exp*v, which
can reach ~7000 because gelu output has a +0.4 mean -> v columns have a DC
component) is normalized to the true `mixed` on the psum->fp8 DVE convert.
The out-proj psum is then 32*(mixed@Wo), folded by a constant 1/1024 on the
fused (psum*c + residual) DVE op; bo and the v-centering correction are
pre-added into the residual on the host.

v2 schedule notes (vs v1):
  - PSUM evictions alternate ACT/DVE so the Vector engine no longer gates
    the projection pipeline (v1: all evicts on DVE lagged ~20us, delaying
    the AllGather staging and everything downstream).
  - Phase 2 runs scores(c0), S(c0), scores(c1), S(c1), mix(c0), out(c0),
    mix(c1), out(c1): each chunk's softmax-denominator reciprocal + DRAM
    partition-broadcast round-trip hides behind the other chunk's scores.
  - kT/v unpack DMAs moved to the gpsimd queue so their AllGather sem-waits
    cannot block the sync queue's staging DMAs.
  - out-proj evict is one fused scalar_tensor_tensor (psum/1024 + residual).
"""

import math

import numpy as np
import ml_dtypes

B, S, D = 4, 2048, 1024
NCORES = 8
SQ = S // 2          # query rows per core
P = 128
DT = 8               # d / 128
KP = DT // 2         # K-pair count for DoubleRow (K=256 each)
ST = S // P          # 16 seq tiles
SQT = SQ // P        # 8
N512 = 512
WARMUP_MMS = 52
WSCALE = 32.0        # host-side weight/bias scale
EXP_BIAS = -5.0 * math.log(2.0)   # expT = exp(scores)/32
EXP_SCALE = 1.0 / 32768.0         # scores_psum = 32768 * scores_true
OUT_DESCALE = 1.0 / 1024.0
GELU_MEAN = 0.3989422804014327    # E[gelu(z)], z ~ N(0,1)

_COMPILED = {}


def _build_program():
    from contextlib import ExitStack

    import concourse.bass as bass
    import concourse.tile as tile
    from concourse import bacc, mybir

    f32 = mybir.dt.float32
    bf16 = mybir.dt.bfloat16
    f8 = mybir.dt.float8e4
    AF = mybir.ActivationFunctionType
    ALU = mybir.AluOpType
    DR = mybir.MatmulPerfMode.DoubleRow

    nc = bacc.Bacc("TRN2", target_bir_lowering=False, debug=False,
                   num_devices=NCORES)

    tokTq = nc.dram_tensor("tokTq", [D, SQ], bf16, kind="ExternalInput")
    resid = nc.dram_tensor("resid", [SQ, D], f32, kind="ExternalInput")
    wq = nc.dram_tensor("wq", [D, D], f8, kind="ExternalInput")
    wk = nc.dram_tensor("wk", [D, D], f8, kind="ExternalInput")
    wv = nc.dram_tensor("wv", [D, D], f8, kind="ExternalInput")
    wo = nc.dram_tensor("wo", [D, D], f8, kind="ExternalInput")
    bq_d = nc.dram_tensor("bq", [D], f32, kind="ExternalInput")   # x32
    bk_d = nc.dram_tensor("bk", [D], f32, kind="ExternalInput")   # x32
    bv_d = nc.dram_tensor("bv", [D], f32, kind="ExternalInput")   # x32
    out_d = nc.dram_tensor("out", [SQ, D], f32, kind="ExternalOutput")

    ts = bass.ts
    groups = [[2 * i, 2 * i + 1] for i in range(NCORES // 2)]

    with tile.TileContext(nc) as tc, ExitStack() as ctx:
        pers = ctx.enter_context(tc.tile_pool(name="pers", bufs=1))
        kT = pers.tile([P, DT, S], f8, tag="kT")
        qT = pers.tile([P, DT, SQ], f8, tag="qT")
        v = pers.tile([P, ST, D], f8, tag="v")
        ones = pers.tile([P, 2, 16], f8, tag="ones")
        bqk = pers.tile([P, 2, DT], f32, tag="bqk")  # [:,0,:]=32bq [:,1,:]=32bk
        ebias = pers.tile([P, 1], f32, tag="ebias")
        wscr = pers.tile([P, N512], bf16, tag="wscr")
        wsink = pers.tile([P, P], f32, tag="wsink")
        wo_sb = pers.tile([P, DT, D], f8, tag="wo")

        dram = ctx.enter_context(tc.tile_pool(name="dram", bufs=1, space="DRAM"))
        kb_in = dram.tile([D, SQ], f8, tag="kb_in")
        kb_out = dram.tile([2, D, SQ], f8, tag="kb_out")
        vb_in = dram.tile([SQ, D], f8, tag="vb_in")
        vb_out = dram.tile([2, SQ, D], f8, tag="vb_out")

        psum = ctx.enter_context(tc.tile_pool(name="psum", bufs=5, space="PSUM"))
        psum_s = ctx.enter_context(tc.tile_pool(name="psum_s", bufs=2, space="PSUM"))

        # --- PE warm-up: dense trivial matmuls so HAM hits K=8/8 and PE is
        # busy while the gelu+DMA head runs.
        nc.vector.memset(wscr, 0.0)
        wps = psum.tile([P, N512], f32, tag="mm")
        for i in range(WARMUP_MMS):
            nc.tensor.matmul(wps, wscr[:, :P], wscr, start=(i == 0),
                             stop=(i == WARMUP_MMS - 1))
        nc.vector.tensor_copy(wsink, wps[:, :P])

        nc.vector.memset(ones, 1.0)
        nc.vector.memset(ebias, EXP_BIAS)
        nc.scalar.dma_start(bqk[:, 0, :], bq_d.ap().rearrange("(t p) -> p t", p=P))
        nc.scalar.dma_start(bqk[:, 1, :], bk_d.ap().rearrange("(t p) -> p t", p=P))

        # ---------------- phase 1: gelu + projections + kT/v exchange -------
        with ExitStack() as ph1:
            p1 = ph1.enter_context(tc.tile_pool(name="p1", bufs=1))
            xTq = p1.tile([P, DT, SQ], f8, tag="xTq")
            wk_sb = p1.tile([P, DT, D], f8, tag="wk")
            wq_sb = p1.tile([P, DT, D], f8, tag="wq")
            wv_sb = p1.tile([P, DT, D], f8, tag="wv")
            bv_sb = p1.tile([P, D], f32, tag="bv")
            kTo = p1.tile([P, DT, SQ], f8, tag="kTo")
            vo = p1.tile([P, SQT, D], f8, tag="vo")
            stag = ph1.enter_context(tc.tile_pool(name="stag", bufs=8))

            # Head is HBM-bound: load ONLY what the gelu needs now (tokens +
            # Wk); Wv/Wq/Wo triggers are interleaved into the staging loops
            # below so their 3MB doesn't steal HBM bandwidth from the tokens.
            nc.gpsimd.dma_start(wk_sb,
                                wk.ap().rearrange("(t p) e -> p t e", p=P))
            nc.gpsimd.dma_start(
                bv_sb, bass.AP(tensor=bv_d, offset=0, ap=[[0, P], [1, D]]))
            # tokens in 4 pair-tiles; gelu per pair so each ACT op unlocks a
            # full DoubleRow K-pair for the projection matmuls
            for g in range(DT // 2):
                stq = stag.tile([P, 2, SQ], bf16, tag="tok", name=f"stq{g}")
                nc.sync.dma_start(
                    stq, tokTq.ap()[2 * g * P:(2 * g + 2) * P, :].rearrange(
                        "(t p) s -> p t s", p=P))
                nc.scalar.activation(xTq[:, 2 * g:2 * g + 2, :], stq, AF.Gelu)

            # kTo: lhsT = Wk-slice, rhs = xTq  -> exchange ASAP. Stream each
            # te row-block to DRAM as its converts land so the AllGather
            # trigger only waits on the last small DMA. The two chunk evicts
            # of each te run on ACT and DVE in parallel.
            kb_in_v = kb_in[:].rearrange("(t p) s -> p t s", p=P)
            for te in range(DT):
                for c in range(SQ // N512):
                    ps = psum.tile([P, N512], f32, tag="mm")
                    for u in range(KP):
                        nc.tensor.matmul(ps, wk_sb[:, 2 * u:2 * u + 2, ts(te, P)],
                                         xTq[:, 2 * u:2 * u + 2, ts(c, N512)],
                                         start=(u == 0), stop=(u == KP - 1),
                                         perf_mode=DR)
                    if c == 0:
                        nc.scalar.activation(kTo[:, te, ts(c, N512)], ps,
                                             AF.Identity, bias=bqk[:, 1, te:te + 1])
                    else:
                        nc.vector.tensor_scalar_add(kTo[:, te, ts(c, N512)], ps,
                                                    bqk[:, 1, te:te + 1])
                nc.sync.dma_start(kb_in_v[:, te, :], kTo[:, te, :])
                if te == 0:
                    nc.sync.dma_start(
                        wv_sb, wv.ap().rearrange("(t p) e -> p t e", p=P))
                elif te == 3:
                    nc.sync.dma_start(
                        wq_sb, wq.ap().rearrange("(t p) e -> p t e", p=P))
            nc.gpsimd.collective_compute(
                "AllGather", mybir.AluOpType.bypass, replica_groups=groups,
                ins=[kb_in[:].opt()], outs=[kb_out[:].opt()])

            # vo : lhsT = xTq-slice, rhs = Wv (own rows)  -> exchange
            vb_in_v = vb_in[:].rearrange("(t p) d -> p t d", p=P)
            for tsq in range(SQT):
                for dc in range(D // N512):
                    ps = psum.tile([P, N512], f32, tag="mm")
                    for u in range(KP):
                        nc.tensor.matmul(ps, xTq[:, 2 * u:2 * u + 2, ts(tsq, P)],
                                         wv_sb[:, 2 * u:2 * u + 2, ts(dc, N512)],
                                         start=(u == 0), stop=(u == KP - 1),
                                         perf_mode=DR)
                    nc.vector.tensor_add(vo[:, tsq, ts(dc, N512)], ps,
                                         bv_sb[:, ts(dc, N512)])
                nc.sync.dma_start(vb_in_v[:, tsq, :], vo[:, tsq, :])
                if tsq == 1:
                    nc.sync.dma_start(
                        wo_sb, wo.ap().rearrange("(t p) e -> p t e", p=P))
            # AG2 trigger BEFORE the kT unpack DMAs on the gpsimd stream:
            # its wire time then starts right after AG1's instead of waiting
            # for the unpacks (which block on AG1 completion).
            nc.gpsimd.collective_compute(
                "AllGather", mybir.AluOpType.bypass, replica_groups=groups,
                ins=[vb_in[:].opt()], outs=[vb_out[:].opt()])
            for r in range(2):
                for hh in range(2):
                    nc.gpsimd.dma_start(
                        kT[:, :, r * SQ + hh * N512:r * SQ + (hh + 1) * N512],
                        kb_out[r][:, hh * N512:(hh + 1) * N512].rearrange(
                            "(t p) s -> p t s", p=P))
            for r in range(2):
                nc.gpsimd.dma_start(
                    v[:, r * SQT:(r + 1) * SQT, :],
                    vb_out[r][:].rearrange("(t p) d -> p t d", p=P))

            # qT : lhsT = Wq-slice, rhs = xTq
            for te in range(DT):
                for c in range(SQ // N512):
                    ps = psum.tile([P, N512], f32, tag="mm")
                    for u in range(KP):
                        nc.tensor.matmul(ps, wq_sb[:, 2 * u:2 * u + 2, ts(te, P)],
                                         xTq[:, 2 * u:2 * u + 2, ts(c, N512)],
                                         start=(u == 0), stop=(u == KP - 1),
                                         perf_mode=DR)
                    if c == 0:
                        nc.scalar.activation(qT[:, te, ts(c, N512)], ps,
                                             AF.Identity, bias=bqk[:, 0, te:te + 1])
                    else:
                        nc.vector.tensor_scalar_add(qT[:, te, ts(c, N512)], ps,
                                                    bqk[:, 0, te:te + 1])

        # ---------------- phase 2: attention + out-proj ----------------
        with ExitStack() as ph2:
            epool = ph2.enter_context(tc.tile_pool(name="ep", bufs=2))
            work = ph2.enter_context(tc.tile_pool(name="wk2", bufs=2))
            opool = ph2.enter_context(tc.tile_pool(name="op2", bufs=2))
            rspool = ph2.enter_context(tc.tile_pool(name="rs2", bufs=2))
            rpool = ph2.enter_context(tc.tile_pool(name="rp", bufs=8))
            dpool = ph2.enter_context(
                tc.tile_pool(name="dram2", bufs=2, space="DRAM"))

            # prefetch all residual rows now (vector queue; consumed by the
            # fused out-proj evict)
            res_sbs = []
            for sl8 in range(SQT):
                res_sb = rpool.tile([P, D], f32, tag="res", name=f"res{sl8}")
                nc.sync.dma_start(res_sb, resid.ap()[sl8 * P:(sl8 + 1) * P, :])
                res_sbs.append(res_sb)

            # scores + softmax denominators for BOTH chunks first: each
            # chunk's reciprocal + DRAM broadcast round-trip hides behind
            # the other chunk's score matmuls.
            expTs, rSbs = [], []
            for c in range(SQ // N512):          # sq chunks of 512
                expT = epool.tile([P, ST, N512], f8, tag="expT",
                                  name=f"expT{c}")
                for tk in range(ST):
                    ps = psum.tile([P, N512], f32, tag="mm")
                    for u in range(KP):
                        nc.tensor.matmul(ps, kT[:, 2 * u:2 * u + 2, ts(tk, P)],
                                         qT[:, 2 * u:2 * u + 2, ts(c, N512)],
                                         start=(u == 0), stop=(u == KP - 1),
                                         perf_mode=DR)
                    nc.scalar.activation(expT[:, tk, :], ps, AF.Exp,
                                         scale=EXP_SCALE, bias=ebias)

                # softmax denominators: ones as 1-col stationary -> S [1, sq]
                psS = psum_s.tile([1, N512], f32, tag="S")
                for tk in range(ST // 2):
                    nc.tensor.matmul(psS, ones[:, :, :1],
                                     expT[:, 2 * tk:2 * tk + 2, :],
                                     start=(tk == 0), stop=(tk == ST // 2 - 1),
                                     perf_mode=DR)
                rS_row = rspool.tile([1, N512], f32, tag="rS_row",
                                     name=f"rS{c}")
                nc.vector.reciprocal(rS_row, psS)   # = 32 / Sigma exp
                # broadcast 1/S across partitions via DRAM (stride-0 DMA;
                # direct SBUF->SBUF partition-scatter DMA mis-writes)
                rs_dram = dpool.tile([N512], f32, tag="rs_dram")
                nc.sync.dma_start(
                    rs_dram[:].rearrange("(o s) -> o s", o=1), rS_row)
                rSb = rspool.tile([P, N512], f32, tag="rSb", name=f"rSb{c}")
                nc.gpsimd.dma_start(rSb, rs_dram[:].partition_broadcast(P))
                expTs.append(expT)
                rSbs.append(rSb)

            for c in range(SQ // N512):
                expT, rSb = expTs[c], rSbs[c]
                # mixedUT[d, sq] = (v^T-stationary @ expT) / S  (normalized on
                # the psum->fp8 convert; unnormalized would overflow e4m3)
                mixUT = work.tile([P, DT, N512], f8, tag="mixUT",
                                  name=f"mixUT{c}")
                for dsl in range(DT):
                    ps = psum.tile([P, N512], f32, tag="mm")
                    for tk in range(ST // 2):
                        nc.tensor.matmul(ps, v[:, 2 * tk:2 * tk + 2, ts(dsl, P)],
                                         expT[:, 2 * tk:2 * tk + 2, :],
                                         start=(tk == 0), stop=(tk == ST // 2 - 1),
                                         perf_mode=DR)
                    nc.vector.tensor_mul(mixUT[:, dsl, :], ps, rSb)

                for sl in range(4):
                    row = (c * 4 + sl) * P
                    res_sb = res_sbs[c * 4 + sl]
                    out_sb = opool.tile([P, D], f32, tag="osb")
                    for ec in range(D // N512):
                        ps = psum.tile([P, N512], f32, tag="mm")
                        for u in range(KP):
                            nc.tensor.matmul(
                                ps, mixUT[:, 2 * u:2 * u + 2, ts(sl, P)],
                                wo_sb[:, 2 * u:2 * u + 2, ts(ec, N512)],
                                start=(u == 0), stop=(u == KP - 1),
                                perf_mode=DR)
                        # out = psum / 1024 + (residual + bo), one fused op
                        nc.vector.scalar_tensor_tensor(
                            out_sb[:, ts(ec, N512)], ps, OUT_DESCALE,
                            res_sb[:, ts(ec, N512)], ALU.mult, ALU.add)
                    nc.sync.dma_start(out_d.ap()[row:row + P, :], out_sb)

    nc.compile()
    return nc


def _get_program():
    if "nc" not in _COMPILED:
        _COMPILED["nc"] = _build_program()
    return _COMPILED["nc"]


def make_in_maps(tokens, Wq, bq, Wk, bk, Wv, bv, Wo, bo):
    tokens = np.asarray(tokens, dtype=np.float32)
    bf = ml_dtypes.bfloat16
    f8 = ml_dtypes.float8_e4m3
    wq_b = np.ascontiguousarray((np.asarray(Wq, np.float32) * WSCALE).astype(f8))
    wk_b = np.ascontiguousarray((np.asarray(Wk, np.float32) * WSCALE).astype(f8))
    wv_b = np.ascontiguousarray((np.asarray(Wv, np.float32) * WSCALE).astype(f8))
    wo_b = np.ascontiguousarray((np.asarray(Wo, np.float32) * WSCALE).astype(f8))
    bq = np.asarray(bq, np.float32) * WSCALE
    bk = np.asarray(bk, np.float32) * WSCALE
    # center v by c ~ E_k[v] so the fp8 mixUT quantizes the small AC part;
    # softmax weights sum to 1, so out = (mixed-c)@Wo + (c@Wo + bo) + resid.
    # bo and the centering correction are folded into the residual host-side.
    wv32 = np.asarray(Wv, np.float32)
    cvec = GELU_MEAN * wv32.sum(axis=0) + np.asarray(bv, np.float32)
    bv = (np.asarray(bv, np.float32) - cvec) * WSCALE
    bo_eff = (np.asarray(bo, np.float32)
              + cvec @ np.asarray(Wo, np.float32)).astype(np.float32)

    in_maps = []
    for c in range(NCORES):
        b, h = divmod(c, 2)
        q_rows = tokens[b, h * SQ:(h + 1) * SQ]
        in_maps.append({
            "tokTq": np.ascontiguousarray(q_rows.T.astype(bf)),  # [D, SQ]
            "resid": np.ascontiguousarray(q_rows + bo_eff),      # [SQ, D] f32
            "wq": wq_b, "wk": wk_b, "wv": wv_b, "wo": wo_b,
            "bq": bq, "bk": bk, "bv": bv,
        })
    return in_maps


def gather_out(results):
    out = np.empty((B, S, D), np.float32)
    for c in range(NCORES):
        b, h = divmod(c, 2)
        out[b, h * SQ:(h + 1) * SQ] = results[c]["out"]
    return out


def kernel(tokens, Wq, bq, Wk, bk, Wv, bv, Wo, bo):
    from concourse.bass_utils import run_bass_kernel_spmd

    in_maps = make_in_maps(tokens, Wq, bq, Wk, bk, Wv, bv, Wo, bo)
    nc = _get_program()
    res = run_bass_kernel_spmd(nc, in_maps, core_ids=list(range(NCORES)),
                               trace=False)
    return gather_out(res.results)


# revision 14
# speedup vs baseline: 1.0234x; 1.0056x over previous
"""Trainium2 Bass kernel for a single-head attention block (B=4, S=2048, D=1024).

reference:
    x = gelu(tokens); q,k,v = x@W{q,k,v} + b; scores = q@k^T/sqrt(D)
    out = softmax(scores)@v @ Wo + bo + tokens

Sharding: 8 cores = 4 batches x 2 query-halves. Core c=2b+h handles batch b and
query rows [h*1024, (h+1)*1024). Each core computes q/k/v projections for its
own rows only; K^T and V halves are exchanged pairwise via two AllGathers. The
fp32 residual path dominates the output magnitude, so the whole matmul pipeline
runs in fp8-e4m3 with DoubleRow perf mode (K=256 per matmul).

KEY LAYOUT TRICK (v4): softmax is permutation-invariant over the key axis as
long as k and v use the SAME order, so each core keeps its OWN key/value rows
in tiles [0, SQ) of kT/v and the PEER's rows in [SQ, 2*SQ). The projection
evictions write straight into the own half (no copy), and the peer half is
pulled from the AllGather output with a dma_gather whose int16 row indices are
HOST-PROVIDED per-core data (peer slot = 1-h) — the program stays SPMD-uniform
while the own-half scores run with no dependency on the collective at all.

Scales: weights are pre-scaled x32 on the host (sigma~1 in fp8), so stored
q,k,v are 32x true scale. scores_psum = 32768*scores_true -> exp uses
scale=2^-15, bias=-5ln2, giving expT = exp(scores)/32 in fp8. Softmax
denominators via a ones-stationary matmul; rS_row = 1/Sigma exp is broadcast
across partitions via a DRAM round-trip (hidden behind the other chunk's
scores). The mixed psum is normalized on the psum->fp8 DVE convert (v is
centered host-side so the fp8 mixUT quantizes the small AC part). The out-proj
psum is 32*(mixed@Wo)*32, folded by 1/1024 on the fused
(psum*c + residual) DVE op; bo and the centering correction are pre-added into
the residual on the host.

Schedule: PSUM evictions alternate ACT/DVE; PE order is
  warmup | kTo -> AG1 | vo -> AG2 | qT | sc0-own sc0-peer S0 | sc1-own
  sc1-peer S1 | mix0 out0 | mix1 out1
so the AllGather wire+gather latency hides behind qT+own-half scores, and each
chunk's softmax reciprocal round-trip hides behind the other chunk's work.
"""

import math

import numpy as np
import ml_dtypes

B, S, D = 4, 2048, 1024
NCORES = 8
SQ = S // 2          # query rows per core
P = 128
DT = 8               # d / 128
KP = DT // 2         # K-pair count for DoubleRow (K=256 each)
ST = S // P          # 16 seq tiles
SQT = SQ // P        # 8
N512 = 512
WARMUP_MMS = 52
WSCALE = 32.0        # host-side weight/bias scale
EXP_BIAS = -5.0 * math.log(2.0)   # expT = exp(scores)/32
EXP_SCALE = 1.0 / 32768.0         # scores_psum = 32768 * scores_true
OUT_DESCALE = 1.0 / 1024.0
GELU_MEAN = 0.3989422804014327    # E[gelu(z)], z ~ N(0,1)

_COMPILED = {}


def _build_program():
    from contextlib import ExitStack

    import concourse.bass as bass
    import concourse.tile as tile
    from concourse import bacc, mybir

    f32 = mybir.dt.float32
    bf16 = mybir.dt.bfloat16
    f8 = mybir.dt.float8e4
    i16 = mybir.dt.int16
    AF = mybir.ActivationFunctionType
    ALU = mybir.AluOpType
    DR = mybir.MatmulPerfMode.DoubleRow

    nc = bacc.Bacc("TRN2", target_bir_lowering=False, debug=False,
                   num_devices=NCORES)

    tokTq = nc.dram_tensor("tokTq", [D, SQ], bf16, kind="ExternalInput")
    resid = nc.dram_tensor("resid", [SQ, D], f32, kind="ExternalInput")
    wq = nc.dram_tensor("wq", [D, D], f8, kind="ExternalInput")
    wk = nc.dram_tensor("wk", [D, D], f8, kind="ExternalInput")
    wv = nc.dram_tensor("wv", [D, D], f8, kind="ExternalInput")
    wo = nc.dram_tensor("wo", [D, D], f8, kind="ExternalInput")
    bq_d = nc.dram_tensor("bq", [D], f32, kind="ExternalInput")   # x32
    bk_d = nc.dram_tensor("bk", [D], f32, kind="ExternalInput")   # x32
    bv_d = nc.dram_tensor("bv", [D], f32, kind="ExternalInput")   # x32
    gidx_d = nc.dram_tensor("gidx", [P, S // 32], i16, kind="ExternalInput")
    out_d = nc.dram_tensor("out", [SQ, D], f32, kind="ExternalOutput")

    ts = bass.ts
    groups = [[2 * i, 2 * i + 1] for i in range(NCORES // 2)]

    with tile.TileContext(nc) as tc, ExitStack() as ctx:
        pers = ctx.enter_context(tc.tile_pool(name="pers", bufs=1))
        kT = pers.tile([P, DT, SQ], f8, tag="kT")     # own keys
        kTp = pers.tile([P, DT, SQ], f8, tag="kTp")   # peer keys
        qT = pers.tile([P, DT, SQ], f8, tag="qT")
        v = pers.tile([P, SQT, D], f8, tag="v")       # own values
        vp = pers.tile([P, SQT, D], f8, tag="vp")     # peer values
        ones = pers.tile([P, 2, 16], f8, tag="ones")
        bqk = pers.tile([P, 2, DT], f32, tag="bqk")  # [:,0,:]=32bq [:,1,:]=32bk
        ebias = pers.tile([P, 1], f32, tag="ebias")
        wscr = pers.tile([P, N512], bf16, tag="wscr")
        wsink = pers.tile([P, P], f32, tag="wsink")
        wo_sb = pers.tile([P, DT, D], f8, tag="wo")
        gidx = pers.tile([P, S // 32], i16, tag="gidx")

        dram = ctx.enter_context(tc.tile_pool(name="dram", bufs=1, space="DRAM"))
        kb_in = dram.tile([D, SQ], f8, tag="kb_in")
        kb_out = dram.tile([2, D, SQ], f8, tag="kb_out")
        vb_in = dram.tile([SQ, D], f8, tag="vb_in")
        vb_out = dram.tile([2, SQ, D], f8, tag="vb_out")

        psum = ctx.enter_context(tc.tile_pool(name="psum", bufs=5, space="PSUM"))
        psum_s = ctx.enter_context(tc.tile_pool(name="psum_s", bufs=2, space="PSUM"))

        # --- PE warm-up: dense trivial matmuls so HAM hits K=8/8 and PE is
        # busy while the gelu+DMA head runs.
        nc.vector.memset(wscr, 0.0)
        wps = psum.tile([P, N512], f32, tag="mm")
        for i in range(WARMUP_MMS):
            nc.tensor.matmul(wps, wscr[:, :P], wscr, start=(i == 0),
                             stop=(i == WARMUP_MMS - 1))
        nc.vector.tensor_copy(wsink, wps[:, :P])

        nc.vector.memset(ones, 1.0)
        nc.vector.memset(ebias, EXP_BIAS)
        nc.scalar.dma_start(bqk[:, 0, :], bq_d.ap().rearrange("(t p) -> p t", p=P))
        nc.scalar.dma_start(bqk[:, 1, :], bk_d.ap().rearrange("(t p) -> p t", p=P))

        # ---------------- phase 1: gelu + projections + kT/v exchange -------
        with ExitStack() as ph1:
            p1 = ph1.enter_context(tc.tile_pool(name="p1", bufs=1))
            xTq = p1.tile([P, DT, SQ], f8, tag="xTq")
            wk_sb = p1.tile([P, DT, D], f8, tag="wk")
            wq_sb = p1.tile([P, DT, D], f8, tag="wq")
            wv_sb = p1.tile([P, DT, D], f8, tag="wv")
            bv_sb = p1.tile([P, D], f32, tag="bv")
            stag = ph1.enter_context(tc.tile_pool(name="stag", bufs=4))

            # Head is HBM-bound: load ONLY what the gelu needs now (tokens +
            # Wk); Wv/Wq/Wo triggers are interleaved into the staging loops
            # below so their 3MB doesn't steal HBM bandwidth from the tokens.
            nc.gpsimd.dma_start(wk_sb,
                                wk.ap().rearrange("(t p) e -> p t e", p=P))
            nc.gpsimd.dma_start(
                bv_sb, bass.AP(tensor=bv_d, offset=0, ap=[[0, P], [1, D]]))
            nc.gpsimd.dma_start(gidx, gidx_d.ap())
            # tokens in 4 pair-tiles; gelu per pair so each ACT op unlocks a
            # full DoubleRow K-pair for the projection matmuls
            for g in range(DT // 2):
                stq = stag.tile([P, 2, SQ], bf16, tag="tok", name=f"stq{g}")
                nc.sync.dma_start(
                    stq, tokTq.ap()[2 * g * P:(2 * g + 2) * P, :].rearrange(
                        "(t p) s -> p t s", p=P))
                nc.scalar.activation(xTq[:, 2 * g:2 * g + 2, :], stq, AF.Gelu)

            # kTo: lhsT = Wk-slice, rhs = xTq -> write own half of kT
            # directly; stream each te row-block to DRAM as its converts land.
            kb_in_v = kb_in[:].rearrange("(t p) s -> p t s", p=P)
            for te in range(DT):
                for c in range(SQ // N512):
                    ps = psum.tile([P, N512], f32, tag="mm")
                    for u in range(KP):
                        nc.tensor.matmul(ps, wk_sb[:, 2 * u:2 * u + 2, ts(te, P)],
                                         xTq[:, 2 * u:2 * u + 2, ts(c, N512)],
                                         start=(u == 0), stop=(u == KP - 1),
                                         perf_mode=DR)
                    if c == 0:
                        nc.scalar.activation(kT[:, te, ts(c, N512)], ps,
                                             AF.Identity, bias=bqk[:, 1, te:te + 1])
                    else:
                        nc.vector.tensor_scalar_add(kT[:, te, ts(c, N512)], ps,
                                                    bqk[:, 1, te:te + 1])
                nc.sync.dma_start(kb_in_v[:, te, :], kT[:, te, :])
                if te == 0:
                    nc.sync.dma_start(
                        wv_sb, wv.ap().rearrange("(t p) e -> p t e", p=P))
                elif te == 3:
                    nc.sync.dma_start(
                        wq_sb, wq.ap().rearrange("(t p) e -> p t e", p=P))
            nc.gpsimd.collective_compute(
                "AllGather", mybir.AluOpType.bypass, replica_groups=groups,
                ins=[kb_in[:].opt()], outs=[kb_out[:].opt()])

            # vo : lhsT = xTq-slice, rhs = Wv -> own half of v
            vb_in_v = vb_in[:].rearrange("(t p) d -> p t d", p=P)
            for tsq in range(SQT):
                for dc in range(D // N512):
                    ps = psum.tile([P, N512], f32, tag="mm")
                    for u in range(KP):
                        nc.tensor.matmul(ps, xTq[:, 2 * u:2 * u + 2, ts(tsq, P)],
                                         wv_sb[:, 2 * u:2 * u + 2, ts(dc, N512)],
                                         start=(u == 0), stop=(u == KP - 1),
                                         perf_mode=DR)
                    nc.vector.tensor_add(v[:, tsq, ts(dc, N512)], ps,
                                         bv_sb[:, ts(dc, N512)])
                nc.sync.dma_start(vb_in_v[:, tsq, :], v[:, tsq, :])
                if tsq == 1:
                    nc.sync.dma_start(
                        wo_sb, wo.ap().rearrange("(t p) e -> p t e", p=P))
            nc.gpsimd.collective_compute(
                "AllGather", mybir.AluOpType.bypass, replica_groups=groups,
                ins=[vb_in[:].opt()], outs=[vb_out[:].opt()])

            # peer halves: gather rows (peer*1024 + j) of the AllGather
            # outputs straight into tiles [SQ, 2SQ) — idx data is per-core
            nc.gpsimd.dma_gather(kTp[:, :, :],
                                 kb_out[:].rearrange("r d s -> (r d) s"),
                                 gidx[:, :], S // 2, S // 2, SQ)
            nc.gpsimd.dma_gather(vp[:, :, :],
                                 vb_out[:].rearrange("r s d -> (r s) d"),
                                 gidx[:, :], S // 2, S // 2, D)

            # qT : lhsT = Wq-slice, rhs = xTq
            for te in range(DT):
                for c in range(SQ // N512):
                    ps = psum.tile([P, N512], f32, tag="mm")
                    for u in range(KP):
                        nc.tensor.matmul(ps, wq_sb[:, 2 * u:2 * u + 2, ts(te, P)],
                                         xTq[:, 2 * u:2 * u + 2, ts(c, N512)],
                                         start=(u == 0), stop=(u == KP - 1),
                                         perf_mode=DR)
                    if c == 0:
                        nc.scalar.activation(qT[:, te, ts(c, N512)], ps,
                                             AF.Identity, bias=bqk[:, 0, te:te + 1])
                    else:
                        nc.vector.tensor_scalar_add(qT[:, te, ts(c, N512)], ps,
                                                    bqk[:, 0, te:te + 1])

        # ---------------- phase 2: attention + out-proj ----------------
        with ExitStack() as ph2:
            epool = ph2.enter_context(tc.tile_pool(name="ep", bufs=2))
            work = ph2.enter_context(tc.tile_pool(name="wk2", bufs=2))
            opool = ph2.enter_context(tc.tile_pool(name="op2", bufs=2))
            rspool = ph2.enter_context(tc.tile_pool(name="rs2", bufs=2))
            rpool = ph2.enter_context(tc.tile_pool(name="rp", bufs=8))
            dpool = ph2.enter_context(
                tc.tile_pool(name="dram2", bufs=2, space="DRAM"))

            # scores in own-half / peer-half blocks; each chunk's softmax
            # denominator + reciprocal round-trip hides behind later blocks
            expTs, rSbs = [], []
            for c in range(SQ // N512):          # sq chunks of 512
                expT = epool.tile([P, ST, N512], f8, tag="expT",
                                  name=f"expT{c}")
                expTs.append(expT)

            def sc_block(c, tk_lo, tk_hi):
                expT = expTs[c]
                for tk in range(tk_lo, tk_hi):
                    ksrc = kT if tk < SQT else kTp
                    ps = psum.tile([P, N512], f32, tag="mm")
                    for u in range(KP):
                        nc.tensor.matmul(ps,
                                         ksrc[:, 2 * u:2 * u + 2,
                                              ts(tk % SQT, P)],
                                         qT[:, 2 * u:2 * u + 2, ts(c, N512)],
                                         start=(u == 0), stop=(u == KP - 1),
                                         perf_mode=DR)
                    nc.scalar.activation(expT[:, tk, :], ps, AF.Exp,
                                         scale=EXP_SCALE, bias=ebias)

            def s_block(c):
                expT = expTs[c]
                psS = psum_s.tile([1, N512], f32, tag="S")
                for tk in range(ST // 2):
                    nc.tensor.matmul(psS, ones[:, :, :1],
                                     expT[:, 2 * tk:2 * tk + 2, :],
                                     start=(tk == 0), stop=(tk == ST // 2 - 1),
                                     perf_mode=DR)
                rS_row = rspool.tile([1, N512], f32, tag="rS_row",
                                     name=f"rS{c}")
                nc.vector.reciprocal(rS_row, psS)   # = 32 / Sigma exp
                # broadcast 1/S across partitions via DRAM (stride-0 DMA)
                rs_dram = dpool.tile([N512], f32, tag="rs_dram")
                nc.sync.dma_start(
                    rs_dram[:].rearrange("(o s) -> o s", o=1), rS_row)
                rSb = rspool.tile([P, N512], f32, tag="rSb", name=f"rSb{c}")
                nc.scalar.dma_start(rSb, rs_dram[:].partition_broadcast(P))
                rSbs.append(rSb)

            sc_block(0, 0, SQT)        # own keys: no collective dependency
            sc_block(0, SQT, ST)       # peer keys: needs AG1 + gather
            s_block(0)

            # residual prefetch AFTER the k-exchange window so its 4MB of
            # HBM reads don't fight the AllGather wire + gathers
            res_sbs = []
            for sl8 in range(SQT):
                res_sb = rpool.tile([P, D], f32, tag="res", name=f"res{sl8}")
                nc.sync.dma_start(res_sb, resid.ap()[sl8 * P:(sl8 + 1) * P, :])
                res_sbs.append(res_sb)

            sc_block(1, 0, SQT)
            sc_block(1, SQT, ST)
            s_block(1)

            for c in range(SQ // N512):
                expT, rSb = expTs[c], rSbs[c]
                # mixedUT[d, sq] = (v^T-stationary @ expT) / S  (normalized on
                # the psum->fp8 convert; unnormalized would overflow e4m3)
                mixUT = work.tile([P, DT, N512], f8, tag="mixUT",
                                  name=f"mixUT{c}")
                for dsl in range(DT):
                    ps = psum.tile([P, N512], f32, tag="mm")
                    for tk in range(ST // 2):
                        vsrc = v if tk < SQT // 2 else vp
                        nc.tensor.matmul(ps,
                                         vsrc[:, (2 * tk) % SQT:
                                              (2 * tk) % SQT + 2, ts(dsl, P)],
                                         expT[:, 2 * tk:2 * tk + 2, :],
                                         start=(tk == 0), stop=(tk == ST // 2 - 1),
                                         perf_mode=DR)
                    nc.vector.tensor_mul(mixUT[:, dsl, :], ps, rSb)

                for sl in range(4):
                    row = (c * 4 + sl) * P
                    res_sb = res_sbs[c * 4 + sl]
                    out_sb = opool.tile([P, D], f32, tag="osb")
                    for ec in range(D // N512):
                        ps = psum.tile([P, N512], f32, tag="mm")
                        for u in range(KP):
                            nc.tensor.matmul(
                                ps, mixUT[:, 2 * u:2 * u + 2, ts(sl, P)],
                                wo_sb[:, 2 * u:2 * u + 2, ts(ec, N512)],
                                start=(u == 0), stop=(u == KP - 1),
                                perf_mode=DR)
                        # out = psum / 1024 + (residual + bo), one fused op
                        nc.vector.scalar_tensor_tensor(
                            out_sb[:, ts(ec, N512)], ps, OUT_DESCALE,
                            res_sb[:, ts(ec, N512)], ALU.mult, ALU.add)
                    nc.sync.dma_start(out_d.ap()[row:row + P, :], out_sb)

    nc.compile()
    return nc


def _get_program():
    if "nc" not in _COMPILED:
        _COMPILED["nc"] = _build_program()
    return _COMPILED["nc"]


def make_in_maps(tokens, Wq, bq, Wk, bk, Wv, bv, Wo, bo):
    tokens = np.asarray(tokens, dtype=np.float32)
    bf = ml_dtypes.bfloat16
    f8 = ml_dtypes.float8_e4m3
    wq_b = np.ascontiguousarray((np.asarray(Wq, np.float32) * WSCALE).astype(f8))
    wk_b = np.ascontiguousarray((np.asarray(Wk, np.float32) * WSCALE).astype(f8))
    wv_b = np.ascontiguousarray((np.asarray(Wv, np.float32) * WSCALE).astype(f8))
    wo_b = np.ascontiguousarray((np.asarray(Wo, np.float32) * WSCALE).astype(f8))
    bq = np.asarray(bq, np.float32) * WSCALE
    bk = np.asarray(bk, np.float32) * WSCALE
    # center v by c ~ E_k[v] so the fp8 mixUT quantizes the small AC part;
    # softmax weights sum to 1, so out = (mixed-c)@Wo + (c@Wo + bo) + resid.
    wv32 = np.asarray(Wv, np.float32)
    cvec = GELU_MEAN * wv32.sum(axis=0) + np.asarray(bv, np.float32)
    bv = (np.asarray(bv, np.float32) - cvec) * WSCALE
    bo_eff = (np.asarray(bo, np.float32)
              + cvec @ np.asarray(Wo, np.float32)).astype(np.float32)

    pp, mm = np.meshgrid(np.arange(P), np.arange(S // 32), indexing="ij")
    base_idx = (mm * 16 + (pp % 16)).astype(np.int16)   # j = m*16 + lane

    in_maps = []
    for c in range(NCORES):
        b, h = divmod(c, 2)
        q_rows = tokens[b, h * SQ:(h + 1) * SQ]
        gidx = (base_idx + np.int16((1 - h) * SQ))       # peer rows
        in_maps.append({
            "tokTq": np.ascontiguousarray(q_rows.T.astype(bf)),  # [D, SQ]
            "resid": np.ascontiguousarray(q_rows + bo_eff),      # [SQ, D] f32
            "wq": wq_b, "wk": wk_b, "wv": wv_b, "wo": wo_b,
            "bq": bq, "bk": bk, "bv": bv,
            "gidx": np.ascontiguousarray(gidx),
        })
    return in_maps


def gather_out(results):
    out = np.empty((B, S, D), np.float32)
    for c in range(NCORES):
        b, h = divmod(c, 2)
        out[b, h * SQ:(h + 1) * SQ] = results[c]["out"]
    return out


def kernel(tokens, Wq, bq, Wk, bk, Wv, bv, Wo, bo):
    from concourse.bass_utils import run_bass_kernel_spmd

    in_maps = make_in_maps(tokens, Wq, bq, Wk, bk, Wv, bv, Wo, bo)
    nc = _get_program()
    res = run_bass_kernel_spmd(nc, in_maps, core_ids=list(range(NCORES)),
                               trace=False)
    return gather_out(res.results)


# revision 15
# speedup vs baseline: 1.1961x; 1.1688x over previous
"""Trainium2 Bass kernel for a single-head attention block (B=4, S=2048, D=1024).

reference:
    x = gelu(tokens); q,k,v = x@W{q,k,v} + b; scores = q@k^T/sqrt(D)
    out = softmax(scores)@v @ Wo + bo + tokens

Sharding: 8 cores = 4 batches x 2 query-halves. Core c=2b+h handles batch b and
query rows [h*1024, (h+1)*1024). Each core computes q/k/v projections for its
own rows only; K^T and V halves are exchanged pairwise via two AllGathers. The
fp32 residual path dominates the output magnitude, so the whole matmul pipeline
runs in fp8-e4m3 with DoubleRow perf mode (K=256 per matmul).

KEY LAYOUT TRICK (v4): softmax is permutation-invariant over the key axis as
long as k and v use the SAME order, so each core keeps its OWN key/value rows
in tiles [0, SQ) of kT/v and the PEER's rows in [SQ, 2*SQ). The projection
evictions write straight into the own half (no copy), and the peer half is
pulled from the AllGather output with a dma_gather whose int16 row indices are
HOST-PROVIDED per-core data (peer slot = 1-h) — the program stays SPMD-uniform
while the own-half scores run with no dependency on the collective at all.

Scales: weights are pre-scaled x32 on the host (sigma~1 in fp8), so stored
q,k,v are 32x true scale. scores_psum = 32768*scores_true -> exp uses
scale=2^-15, bias=-5ln2, giving expT = exp(scores)/32 in fp8. Softmax
denominators via a ones-stationary matmul; rS_row = 1/Sigma exp is broadcast
across partitions via a DRAM round-trip (hidden behind the other chunk's
scores). The mixed psum is normalized on the psum->fp8 DVE convert (v is
centered host-side so the fp8 mixUT quantizes the small AC part). The out-proj
psum is 32*(mixed@Wo)*32, folded by 1/1024 on the fused
(psum*c + residual) DVE op; bo and the centering correction are pre-added into
the residual on the host.

Schedule: PSUM evictions alternate ACT/DVE; PE order is
  warmup | kTo -> AG1 | vo -> AG2 | qT | sc0-own sc0-peer S0 | sc1-own
  sc1-peer S1 | mix0 out0 | mix1 out1
so the AllGather wire+gather latency hides behind qT+own-half scores, and each
chunk's softmax reciprocal round-trip hides behind the other chunk's work.
"""

import math

import numpy as np
import ml_dtypes

B, S, D = 4, 2048, 1024
NCORES = 8
SQ = S // 2          # query rows per core
P = 128
DT = 8               # d / 128
KP = DT // 2         # K-pair count for DoubleRow (K=256 each)
ST = S // P          # 16 seq tiles
SQT = SQ // P        # 8
N512 = 512
WARMUP_MMS = 12
WSCALE = 32.0        # host-side weight/bias scale
EXP_BIAS = -5.0 * math.log(2.0)   # expT = exp(scores)/32
EXP_SCALE = 1.0 / 32768.0         # scores_psum = 32768 * scores_true
OUT_DESCALE = 1.0 / 1024.0
GELU_MEAN = 0.3989422804014327    # E[gelu(z)], z ~ N(0,1)

_COMPILED = {}


def _build_program():
    from contextlib import ExitStack

    import concourse.bass as bass
    import concourse.tile as tile
    from concourse import bacc, mybir

    f32 = mybir.dt.float32
    bf16 = mybir.dt.bfloat16
    f8 = mybir.dt.float8e4
    i16 = mybir.dt.int16
    AF = mybir.ActivationFunctionType
    ALU = mybir.AluOpType
    DR = mybir.MatmulPerfMode.DoubleRow

    nc = bacc.Bacc("TRN2", target_bir_lowering=False, debug=False,
                   num_devices=NCORES)

    tokTq = nc.dram_tensor("tokTq", [D, SQ], bf16, kind="ExternalInput")
    resid = nc.dram_tensor("resid", [SQ, D], f32, kind="ExternalInput")
    wq = nc.dram_tensor("wq", [D, D], f8, kind="ExternalInput")
    wk = nc.dram_tensor("wk", [D, D], f8, kind="ExternalInput")
    wv = nc.dram_tensor("wv", [D, D], f8, kind="ExternalInput")
    wo = nc.dram_tensor("wo", [D, D], f8, kind="ExternalInput")
    bq_d = nc.dram_tensor("bq", [D], f32, kind="ExternalInput")   # x32
    bk_d = nc.dram_tensor("bk", [D], f32, kind="ExternalInput")   # x32
    bv_d = nc.dram_tensor("bv", [D], f32, kind="ExternalInput")   # x32
    gidxk_d = nc.dram_tensor("gidxk", [P, S // 64], i16, kind="ExternalInput")
    gidxv_d = nc.dram_tensor("gidxv", [P, S // 32], i16, kind="ExternalInput")
    out_d = nc.dram_tensor("out", [SQ, D], f32, kind="ExternalOutput")

    ts = bass.ts
    groups = [[2 * i, 2 * i + 1] for i in range(NCORES // 2)]

    with tile.TileContext(nc) as tc, ExitStack() as ctx:
        pers = ctx.enter_context(tc.tile_pool(name="pers", bufs=1))
        kT = pers.tile([P, DT, SQ], f8, tag="kT")     # own keys
        kTp = pers.tile([P, DT, SQ], f8, tag="kTp")   # peer keys
        qT = pers.tile([P, DT, SQ], f8, tag="qT")
        v = pers.tile([P, SQT, D], f8, tag="v")       # own values
        vp = pers.tile([P, SQT, D], f8, tag="vp")     # peer values
        ones = pers.tile([P, 2, 16], f8, tag="ones")
        bqk = pers.tile([P, 2, DT], f32, tag="bqk")  # [:,0,:]=32bq [:,1,:]=32bk
        ebias = pers.tile([P, 1], f32, tag="ebias")
        wscr = pers.tile([P, N512], bf16, tag="wscr")
        wsink = pers.tile([P, P], f32, tag="wsink")
        wo_sb = pers.tile([P, DT, D], f8, tag="wo")
        gidxk = pers.tile([P, S // 64], i16, tag="gidxk")
        gidxv = pers.tile([P, S // 32], i16, tag="gidxv")

        dram = ctx.enter_context(tc.tile_pool(name="dram", bufs=1, space="DRAM"))
        kb_in_a = dram.tile([D // 2, SQ], f8, tag="kb_in_a")
        kb_in_b = dram.tile([D // 2, SQ], f8, tag="kb_in_b")
        kb_out_a = dram.tile([2, D // 2, SQ], f8, tag="kb_out_a")
        kb_out_b = dram.tile([2, D // 2, SQ], f8, tag="kb_out_b")
        vb_in = dram.tile([SQ, D], f8, tag="vb_in")
        vb_out = dram.tile([2, SQ, D], f8, tag="vb_out")

        psum = ctx.enter_context(tc.tile_pool(name="psum", bufs=5, space="PSUM"))
        psum_s = ctx.enter_context(tc.tile_pool(name="psum_s", bufs=2, space="PSUM"))

        # --- PE warm-up: dense trivial matmuls so HAM hits K=8/8 and PE is
        # busy while the gelu+DMA head runs.
        nc.vector.memset(wscr, 0.0)
        wps = psum.tile([P, N512], f32, tag="mm")
        for i in range(WARMUP_MMS):
            nc.tensor.matmul(wps, wscr[:, :P], wscr, start=(i == 0),
                             stop=(i == WARMUP_MMS - 1))
        nc.vector.tensor_copy(wsink, wps[:, :P])

        nc.vector.memset(ones, 1.0)
        nc.vector.memset(ebias, EXP_BIAS)
        nc.scalar.dma_start(bqk[:, 0, :], bq_d.ap().rearrange("(t p) -> p t", p=P))
        nc.scalar.dma_start(bqk[:, 1, :], bk_d.ap().rearrange("(t p) -> p t", p=P))

        # ---------------- phase 1: gelu + projections + kT/v exchange -------
        with ExitStack() as ph1:
            p1 = ph1.enter_context(tc.tile_pool(name="p1", bufs=1))
            xTq = p1.tile([P, DT, SQ], f8, tag="xTq")
            wk_sb = p1.tile([P, DT, D], f8, tag="wk")
            wq_sb = p1.tile([P, DT, D], f8, tag="wq")
            wv_sb = p1.tile([P, DT, D], f8, tag="wv")
            bv_sb = p1.tile([P, D], f32, tag="bv")
            stag = ph1.enter_context(tc.tile_pool(name="stag", bufs=4))

            # Head is HBM-bound: load ONLY what the gelu needs now (tokens +
            # Wk); Wv/Wq/Wo triggers are interleaved into the staging loops
            # below so their 3MB doesn't steal HBM bandwidth from the tokens.
            nc.gpsimd.dma_start(wk_sb,
                                wk.ap().rearrange("(t p) e -> p t e", p=P))
            nc.gpsimd.dma_start(
                bv_sb, bass.AP(tensor=bv_d, offset=0, ap=[[0, P], [1, D]]))
            nc.gpsimd.dma_start(gidxk, gidxk_d.ap())
            nc.gpsimd.dma_start(gidxv, gidxv_d.ap())
            # tokens in 4 pair-tiles; gelu per pair so each ACT op unlocks a
            # full DoubleRow K-pair for the projection matmuls
            for g in range(DT // 2):
                stq = stag.tile([P, 2, SQ], bf16, tag="tok", name=f"stq{g}")
                nc.sync.dma_start(
                    stq, tokTq.ap()[2 * g * P:(2 * g + 2) * P, :].rearrange(
                        "(t p) s -> p t s", p=P))
                nc.scalar.activation(xTq[:, 2 * g:2 * g + 2, :], stq, AF.Gelu)

            # kTo: lhsT = Wk-slice, rhs = xTq -> write own half of kT
            # directly; stream each te row-block to DRAM as its converts
            # land. The exchange is split into two half-AllGathers so the
            # first fires as soon as te 0-3 are staged.
            kb_in_av = kb_in_a[:].rearrange("(t p) s -> p t s", p=P)
            kb_in_bv = kb_in_b[:].rearrange("(t p) s -> p t s", p=P)
            for te in range(DT):
                for c in range(SQ // N512):
                    ps = psum.tile([P, N512], f32, tag="mm")
                    for u in range(KP):
                        nc.tensor.matmul(ps, wk_sb[:, 2 * u:2 * u + 2, ts(te, P)],
                                         xTq[:, 2 * u:2 * u + 2, ts(c, N512)],
                                         start=(u == 0), stop=(u == KP - 1),
                                         perf_mode=DR)
                    if c == 0:
                        nc.scalar.activation(kT[:, te, ts(c, N512)], ps,
                                             AF.Identity, bias=bqk[:, 1, te:te + 1])
                    else:
                        nc.vector.tensor_scalar_add(kT[:, te, ts(c, N512)], ps,
                                                    bqk[:, 1, te:te + 1])
                kb_v = kb_in_av if te < 4 else kb_in_bv
                nc.sync.dma_start(kb_v[:, te % 4, :], kT[:, te, :])
                if te == 0:
                    nc.sync.dma_start(
                        wv_sb, wv.ap().rearrange("(t p) e -> p t e", p=P))
                elif te == 3:
                    nc.sync.dma_start(
                        wq_sb, wq.ap().rearrange("(t p) e -> p t e", p=P))
                elif te == DT - 1:
                    pass
                if te == 3:
                    nc.gpsimd.collective_compute(
                        "AllGather", mybir.AluOpType.bypass,
                        replica_groups=groups,
                        ins=[kb_in_a[:].opt()], outs=[kb_out_a[:].opt()])
            nc.gpsimd.collective_compute(
                "AllGather", mybir.AluOpType.bypass, replica_groups=groups,
                ins=[kb_in_b[:].opt()], outs=[kb_out_b[:].opt()])

            # vo : lhsT = xTq-slice, rhs = Wv -> own half of v
            vb_in_v = vb_in[:].rearrange("(t p) d -> p t d", p=P)
            for tsq in range(SQT):
                for dc in range(D // N512):
                    ps = psum.tile([P, N512], f32, tag="mm")
                    for u in range(KP):
                        nc.tensor.matmul(ps, xTq[:, 2 * u:2 * u + 2, ts(tsq, P)],
                                         wv_sb[:, 2 * u:2 * u + 2, ts(dc, N512)],
                                         start=(u == 0), stop=(u == KP - 1),
                                         perf_mode=DR)
                    nc.vector.tensor_add(v[:, tsq, ts(dc, N512)], ps,
                                         bv_sb[:, ts(dc, N512)])
                nc.sync.dma_start(vb_in_v[:, tsq, :], v[:, tsq, :])
                if tsq == 1:
                    nc.sync.dma_start(
                        wo_sb, wo.ap().rearrange("(t p) e -> p t e", p=P))
            nc.gpsimd.collective_compute(
                "AllGather", mybir.AluOpType.bypass, replica_groups=groups,
                ins=[vb_in[:].opt()], outs=[vb_out[:].opt()])

            # peer halves: gather the peer's rows of the AllGather outputs
            # straight into the peer tiles — idx data is per-core
            nc.gpsimd.dma_gather(kTp[:, 0:4, :],
                                 kb_out_a[:].rearrange("r d s -> (r d) s"),
                                 gidxk[:, :], S // 4, S // 4, SQ)
            nc.gpsimd.dma_gather(kTp[:, 4:8, :],
                                 kb_out_b[:].rearrange("r d s -> (r d) s"),
                                 gidxk[:, :], S // 4, S // 4, SQ)
            nc.gpsimd.dma_gather(vp[:, :, :],
                                 vb_out[:].rearrange("r s d -> (r s) d"),
                                 gidxv[:, :], S // 2, S // 2, D)

            # qT : lhsT = Wq-slice, rhs = xTq
            for te in range(DT):
                for c in range(SQ // N512):
                    ps = psum.tile([P, N512], f32, tag="mm")
                    for u in range(KP):
                        nc.tensor.matmul(ps, wq_sb[:, 2 * u:2 * u + 2, ts(te, P)],
                                         xTq[:, 2 * u:2 * u + 2, ts(c, N512)],
                                         start=(u == 0), stop=(u == KP - 1),
                                         perf_mode=DR)
                    if c == 0:
                        nc.scalar.activation(qT[:, te, ts(c, N512)], ps,
                                             AF.Identity, bias=bqk[:, 0, te:te + 1])
                    else:
                        nc.vector.tensor_scalar_add(qT[:, te, ts(c, N512)], ps,
                                                    bqk[:, 0, te:te + 1])

        # ---------------- phase 2: attention + out-proj ----------------
        with ExitStack() as ph2:
            epool = ph2.enter_context(tc.tile_pool(name="ep", bufs=2))
            work = ph2.enter_context(tc.tile_pool(name="wk2", bufs=2))
            opool = ph2.enter_context(tc.tile_pool(name="op2", bufs=2))
            rspool = ph2.enter_context(tc.tile_pool(name="rs2", bufs=2))
            rpool = ph2.enter_context(tc.tile_pool(name="rp", bufs=8))
            dpool = ph2.enter_context(
                tc.tile_pool(name="dram2", bufs=2, space="DRAM"))

            # scores in own-half / peer-half blocks; each chunk's softmax
            # denominator + reciprocal round-trip hides behind later blocks
            expTs, rSbs = [], []
            for c in range(SQ // N512):          # sq chunks of 512
                expT = epool.tile([P, ST, N512], f8, tag="expT",
                                  name=f"expT{c}")
                expTs.append(expT)

            def sc_block(c, tk_lo, tk_hi):
                expT = expTs[c]
                for tk in range(tk_lo, tk_hi):
                    ksrc = kT if tk < SQT else kTp
                    ps = psum.tile([P, N512], f32, tag="mm")
                    for u in range(KP):
                        nc.tensor.matmul(ps,
                                         ksrc[:, 2 * u:2 * u + 2,
                                              ts(tk % SQT, P)],
                                         qT[:, 2 * u:2 * u + 2, ts(c, N512)],
                                         start=(u == 0), stop=(u == KP - 1),
                                         perf_mode=DR)
                    nc.scalar.activation(expT[:, tk, :], ps, AF.Exp,
                                         scale=EXP_SCALE, bias=ebias)

            def s_block(c):
                expT = expTs[c]
                psS = psum_s.tile([1, N512], f32, tag="S")
                for tk in range(ST // 2):
                    nc.tensor.matmul(psS, ones[:, :, :1],
                                     expT[:, 2 * tk:2 * tk + 2, :],
                                     start=(tk == 0), stop=(tk == ST // 2 - 1),
                                     perf_mode=DR)
                rS_row = rspool.tile([1, N512], f32, tag="rS_row",
                                     name=f"rS{c}")
                nc.vector.reciprocal(rS_row, psS)   # = 32 / Sigma exp
                # broadcast 1/S across partitions via DRAM (stride-0 DMA)
                rs_dram = dpool.tile([N512], f32, tag="rs_dram")
                nc.sync.dma_start(
                    rs_dram[:].rearrange("(o s) -> o s", o=1), rS_row)
                rSb = rspool.tile([P, N512], f32, tag="rSb", name=f"rSb{c}")
                nc.scalar.dma_start(rSb, rs_dram[:].partition_broadcast(P))
                rSbs.append(rSb)

            sc_block(0, 0, SQT)        # own keys: no collective dependency
            sc_block(1, 0, SQT)        # more own-key work to hide the wire
            sc_block(0, SQT, ST)       # peer keys: needs AG1 + gathers
            s_block(0)

            # residual prefetch AFTER the k-exchange window so its 4MB of
            # HBM reads don't fight the AllGather wire + gathers
            res_sbs = []
            for sl8 in range(SQT):
                res_sb = rpool.tile([P, D], f32, tag="res", name=f"res{sl8}")
                nc.sync.dma_start(res_sb, resid.ap()[sl8 * P:(sl8 + 1) * P, :])
                res_sbs.append(res_sb)

            sc_block(1, SQT, ST)
            s_block(1)

            for c in range(SQ // N512):
                expT, rSb = expTs[c], rSbs[c]
                # mixedUT[d, sq] = (v^T-stationary @ expT) / S  (normalized on
                # the psum->fp8 convert; unnormalized would overflow e4m3)
                mixUT = work.tile([P, DT, N512], f8, tag="mixUT",
                                  name=f"mixUT{c}")
                for dsl in range(DT):
                    ps = psum.tile([P, N512], f32, tag="mm")
                    for tk in range(ST // 2):
                        vsrc = v if tk < SQT // 2 else vp
                        nc.tensor.matmul(ps,
                                         vsrc[:, (2 * tk) % SQT:
                                              (2 * tk) % SQT + 2, ts(dsl, P)],
                                         expT[:, 2 * tk:2 * tk + 2, :],
                                         start=(tk == 0), stop=(tk == ST // 2 - 1),
                                         perf_mode=DR)
                    nc.vector.tensor_mul(mixUT[:, dsl, :], ps, rSb)

                for sl in range(4):
                    row = (c * 4 + sl) * P
                    res_sb = res_sbs[c * 4 + sl]
                    out_sb = opool.tile([P, D], f32, tag="osb")
                    for ec in range(D // N512):
                        ps = psum.tile([P, N512], f32, tag="mm")
                        for u in range(KP):
                            nc.tensor.matmul(
                                ps, mixUT[:, 2 * u:2 * u + 2, ts(sl, P)],
                                wo_sb[:, 2 * u:2 * u + 2, ts(ec, N512)],
                                start=(u == 0), stop=(u == KP - 1),
                                perf_mode=DR)
                        # out = psum / 1024 + (residual + bo), one fused op
                        nc.vector.scalar_tensor_tensor(
                            out_sb[:, ts(ec, N512)], ps, OUT_DESCALE,
                            res_sb[:, ts(ec, N512)], ALU.mult, ALU.add)
                    nc.sync.dma_start(out_d.ap()[row:row + P, :], out_sb)

    nc.compile()
    return nc


def _get_program():
    if "nc" not in _COMPILED:
        _COMPILED["nc"] = _build_program()
    return _COMPILED["nc"]


def make_in_maps(tokens, Wq, bq, Wk, bk, Wv, bv, Wo, bo):
    tokens = np.asarray(tokens, dtype=np.float32)
    bf = ml_dtypes.bfloat16
    f8 = ml_dtypes.float8_e4m3
    wq_b = np.ascontiguousarray((np.asarray(Wq, np.float32) * WSCALE).astype(f8))
    wk_b = np.ascontiguousarray((np.asarray(Wk, np.float32) * WSCALE).astype(f8))
    wv_b = np.ascontiguousarray((np.asarray(Wv, np.float32) * WSCALE).astype(f8))
    wo_b = np.ascontiguousarray((np.asarray(Wo, np.float32) * WSCALE).astype(f8))
    bq = np.asarray(bq, np.float32) * WSCALE
    bk = np.asarray(bk, np.float32) * WSCALE
    # center v by c ~ E_k[v] so the fp8 mixUT quantizes the small AC part;
    # softmax weights sum to 1, so out = (mixed-c)@Wo + (c@Wo + bo) + resid.
    wv32 = np.asarray(Wv, np.float32)
    cvec = GELU_MEAN * wv32.sum(axis=0) + np.asarray(bv, np.float32)
    bv = (np.asarray(bv, np.float32) - cvec) * WSCALE
    bo_eff = (np.asarray(bo, np.float32)
              + cvec @ np.asarray(Wo, np.float32)).astype(np.float32)

    pp, mm = np.meshgrid(np.arange(P), np.arange(S // 32), indexing="ij")
    base_v = (mm * 16 + (pp % 16)).astype(np.int16)     # j = m*16 + lane
    base_k = base_v[:, :S // 64]

    in_maps = []
    for c in range(NCORES):
        b, h = divmod(c, 2)
        q_rows = tokens[b, h * SQ:(h + 1) * SQ]
        in_maps.append({
            "tokTq": np.ascontiguousarray(q_rows.T.astype(bf)),  # [D, SQ]
            "resid": np.ascontiguousarray(q_rows + bo_eff),      # [SQ, D] f32
            "wq": wq_b, "wk": wk_b, "wv": wv_b, "wo": wo_b,
            "bq": bq, "bk": bk, "bv": bv,
            "gidxk": np.ascontiguousarray(base_k + np.int16((1 - h) * (SQ // 2))),
            "gidxv": np.ascontiguousarray(base_v + np.int16((1 - h) * SQ)),
        })
    return in_maps


def gather_out(results):
    out = np.empty((B, S, D), np.float32)
    for c in range(NCORES):
        b, h = divmod(c, 2)
        out[b, h * SQ:(h + 1) * SQ] = results[c]["out"]
    return out


def kernel(tokens, Wq, bq, Wk, bk, Wv, bv, Wo, bo):
    from concourse.bass_utils import run_bass_kernel_spmd

    in_maps = make_in_maps(tokens, Wq, bq, Wk, bk, Wv, bv, Wo, bo)
    nc = _get_program()
    res = run_bass_kernel_spmd(nc, in_maps, core_ids=list(range(NCORES)),
                               trace=False)
    return gather_out(res.results)


# revision 16
# speedup vs baseline: 1.2320x; 1.0300x over previous
"""Trainium2 Bass kernel for a single-head attention block (B=4, S=2048, D=1024).

reference:
    x = gelu(tokens); q,k,v = x@W{q,k,v} + b; scores = q@k^T/sqrt(D)
    out = softmax(scores)@v @ Wo + bo + tokens

Sharding: 8 cores = 4 batches x 2 query-halves. Core c=2b+h handles batch b and
query rows [h*1024, (h+1)*1024). Each core computes q/k/v projections for its
own rows only; K^T and V halves are exchanged pairwise via two AllGathers. The
fp32 residual path dominates the output magnitude, so the whole matmul pipeline
runs in fp8-e4m3 with DoubleRow perf mode (K=256 per matmul).

KEY LAYOUT TRICK (v4): softmax is permutation-invariant over the key axis as
long as k and v use the SAME order, so each core keeps its OWN key/value rows
in tiles [0, SQ) of kT/v and the PEER's rows in [SQ, 2*SQ). The projection
evictions write straight into the own half (no copy), and the peer half is
pulled from the AllGather output with a dma_gather whose int16 row indices are
HOST-PROVIDED per-core data (peer slot = 1-h) — the program stays SPMD-uniform
while the own-half scores run with no dependency on the collective at all.

Scales: weights are pre-scaled x32 on the host (sigma~1 in fp8), so stored
q,k,v are 32x true scale. scores_psum = 32768*scores_true -> exp uses
scale=2^-15, bias=-5ln2, giving expT = exp(scores)/32 in fp8. Softmax
denominators via a ones-stationary matmul; rS_row = 1/Sigma exp is broadcast
across partitions via a DRAM round-trip (hidden behind the other chunk's
scores). The mixed psum is normalized on the psum->fp8 DVE convert (v is
centered host-side so the fp8 mixUT quantizes the small AC part). The out-proj
psum is 32*(mixed@Wo)*32, folded by 1/1024 on the fused
(psum*c + residual) DVE op; bo and the centering correction are pre-added into
the residual on the host.

Schedule: PSUM evictions alternate ACT/DVE; PE order is
  warmup | kTo -> AG1 | vo -> AG2 | qT | sc0-own sc0-peer S0 | sc1-own
  sc1-peer S1 | mix0 out0 | mix1 out1
so the AllGather wire+gather latency hides behind qT+own-half scores, and each
chunk's softmax reciprocal round-trip hides behind the other chunk's work.
"""

import math

import numpy as np
import ml_dtypes

B, S, D = 4, 2048, 1024
NCORES = 8
SQ = S // 2          # query rows per core
P = 128
DT = 8               # d / 128
KP = DT // 2         # K-pair count for DoubleRow (K=256 each)
ST = S // P          # 16 seq tiles
SQT = SQ // P        # 8
N512 = 512
WARMUP_MMS = 20
WSCALE = 32.0        # host-side weight/bias scale
EXP_BIAS = -5.0 * math.log(2.0)   # expT = exp(scores)/32
EXP_SCALE = 1.0 / 32768.0         # scores_psum = 32768 * scores_true
OUT_DESCALE = 1.0 / 1024.0
GELU_MEAN = 0.3989422804014327    # E[gelu(z)], z ~ N(0,1)

_COMPILED = {}


def _build_program():
    from contextlib import ExitStack

    import concourse.bass as bass
    import concourse.tile as tile
    from concourse import bacc, mybir

    f32 = mybir.dt.float32
    bf16 = mybir.dt.bfloat16
    f8 = mybir.dt.float8e4
    i16 = mybir.dt.int16
    AF = mybir.ActivationFunctionType
    ALU = mybir.AluOpType
    DR = mybir.MatmulPerfMode.DoubleRow

    nc = bacc.Bacc("TRN2", target_bir_lowering=False, debug=False,
                   num_devices=NCORES)

    tokTq = nc.dram_tensor("tokTq", [D, SQ], bf16, kind="ExternalInput")
    resid = nc.dram_tensor("resid", [SQ, D], bf16, kind="ExternalInput")
    wq = nc.dram_tensor("wq", [D, D], f8, kind="ExternalInput")
    wk = nc.dram_tensor("wk", [D, D], f8, kind="ExternalInput")
    wv = nc.dram_tensor("wv", [D, D], f8, kind="ExternalInput")
    wo = nc.dram_tensor("wo", [D, D], f8, kind="ExternalInput")
    bq_d = nc.dram_tensor("bq", [D], f32, kind="ExternalInput")   # x32
    bk_d = nc.dram_tensor("bk", [D], f32, kind="ExternalInput")   # x32
    bv_d = nc.dram_tensor("bv", [D], f32, kind="ExternalInput")   # x32
    gidxk_d = nc.dram_tensor("gidxk", [P, S // 64], i16, kind="ExternalInput")
    gidxv_d = nc.dram_tensor("gidxv", [P, S // 32], i16, kind="ExternalInput")
    out_d = nc.dram_tensor("out", [SQ, D], f32, kind="ExternalOutput")

    ts = bass.ts
    groups = [[2 * i, 2 * i + 1] for i in range(NCORES // 2)]

    with tile.TileContext(nc) as tc, ExitStack() as ctx:
        pers = ctx.enter_context(tc.tile_pool(name="pers", bufs=1))
        kT = pers.tile([P, DT, SQ], f8, tag="kT")     # own keys
        kTp = pers.tile([P, DT, SQ], f8, tag="kTp")   # peer keys
        qT = pers.tile([P, DT, SQ], f8, tag="qT")
        v = pers.tile([P, SQT, D], f8, tag="v")       # own values
        vp = pers.tile([P, SQT, D], f8, tag="vp")     # peer values
        ones = pers.tile([P, 2, 16], f8, tag="ones")
        bqk = pers.tile([P, 2, DT], f32, tag="bqk")  # [:,0,:]=32bq [:,1,:]=32bk
        ebias = pers.tile([P, 1], f32, tag="ebias")
        wscr = pers.tile([P, N512], bf16, tag="wscr")
        wsink = pers.tile([P, P], f32, tag="wsink")
        wo_sb = pers.tile([P, DT, D], f8, tag="wo")
        gidxk = pers.tile([P, S // 64], i16, tag="gidxk")
        gidxv = pers.tile([P, S // 32], i16, tag="gidxv")

        dram = ctx.enter_context(tc.tile_pool(name="dram", bufs=1, space="DRAM"))
        bar_in = dram.tile([64], f8, tag="bar_in")
        bar_out = dram.tile([2, 64], f8, tag="bar_out")
        kb_in_a = dram.tile([D // 2, SQ], f8, tag="kb_in_a")
        kb_in_b = dram.tile([D // 2, SQ], f8, tag="kb_in_b")
        kb_out_a = dram.tile([2, D // 2, SQ], f8, tag="kb_out_a")
        kb_out_b = dram.tile([2, D // 2, SQ], f8, tag="kb_out_b")
        vb_in = dram.tile([SQ, D], f8, tag="vb_in")
        vb_out = dram.tile([2, SQ, D], f8, tag="vb_out")

        psum = ctx.enter_context(tc.tile_pool(name="psum", bufs=5, space="PSUM"))
        psum_s = ctx.enter_context(tc.tile_pool(name="psum_s", bufs=2, space="PSUM"))

        # --- PE warm-up: dense trivial matmuls so HAM hits K=8/8 and PE is
        # busy while the gelu+DMA head runs.
        nc.vector.memset(wscr, 0.0)
        wps = psum.tile([P, N512], f32, tag="mm")
        for i in range(WARMUP_MMS):
            nc.tensor.matmul(wps, wscr[:, :P], wscr, start=(i == 0),
                             stop=(i == WARMUP_MMS - 1))
        nc.vector.tensor_copy(wsink, wps[:, :P])

        nc.vector.memset(ones, 1.0)
        nc.vector.memset(ebias, EXP_BIAS)
        nc.scalar.dma_start(bqk[:, 0, :], bq_d.ap().rearrange("(t p) -> p t", p=P))
        nc.scalar.dma_start(bqk[:, 1, :], bk_d.ap().rearrange("(t p) -> p t", p=P))

        # ---------------- phase 1: gelu + projections + kT/v exchange -------
        with ExitStack() as ph1:
            p1 = ph1.enter_context(tc.tile_pool(name="p1", bufs=1))
            xTq = p1.tile([P, DT, SQ], f8, tag="xTq")
            wk_sb = p1.tile([P, DT, D], f8, tag="wk")
            wq_sb = p1.tile([P, DT, D], f8, tag="wq")
            wv_sb = p1.tile([P, DT, D], f8, tag="wv")
            bv_sb = p1.tile([P, D], f32, tag="bv")
            stag = ph1.enter_context(tc.tile_pool(name="stag", bufs=4))

            # Head is HBM-bound: load ONLY what the gelu needs now (tokens +
            # Wk); Wv/Wq/Wo triggers are interleaved into the staging loops
            # below so their 3MB doesn't steal HBM bandwidth from the tokens.
            nc.gpsimd.dma_start(wk_sb,
                                wk.ap().rearrange("(t p) e -> p t e", p=P))
            nc.gpsimd.dma_start(
                bv_sb, bass.AP(tensor=bv_d, offset=0, ap=[[0, P], [1, D]]))
            nc.gpsimd.dma_start(gidxk, gidxk_d.ap())
            nc.gpsimd.dma_start(gidxv, gidxv_d.ap())
            # tiny pair-barrier during the gelu head: absorbs the cross-core
            # launch skew so the first real AllGather's wire starts promptly
            nc.gpsimd.collective_compute(
                "AllGather", mybir.AluOpType.bypass, replica_groups=groups,
                ins=[bar_in[:].opt()], outs=[bar_out[:].opt()])
            # tokens in 4 pair-tiles; gelu per pair so each ACT op unlocks a
            # full DoubleRow K-pair for the projection matmuls
            for g in range(DT // 2):
                stq = stag.tile([P, 2, SQ], bf16, tag="tok", name=f"stq{g}")
                nc.sync.dma_start(
                    stq, tokTq.ap()[2 * g * P:(2 * g + 2) * P, :].rearrange(
                        "(t p) s -> p t s", p=P))
                nc.scalar.activation(xTq[:, 2 * g:2 * g + 2, :], stq, AF.Gelu)

            # kTo: lhsT = Wk-slice, rhs = xTq -> write own half of kT
            # directly; stream each te row-block to DRAM as its converts
            # land. The exchange is split into two half-AllGathers so the
            # first fires as soon as te 0-3 are staged.
            kb_in_av = kb_in_a[:].rearrange("(t p) s -> p t s", p=P)
            kb_in_bv = kb_in_b[:].rearrange("(t p) s -> p t s", p=P)
            for te in range(DT):
                for c in range(SQ // N512):
                    ps = psum.tile([P, N512], f32, tag="mm")
                    for u in range(KP):
                        nc.tensor.matmul(ps, wk_sb[:, 2 * u:2 * u + 2, ts(te, P)],
                                         xTq[:, 2 * u:2 * u + 2, ts(c, N512)],
                                         start=(u == 0), stop=(u == KP - 1),
                                         perf_mode=DR)
                    if c == 0:
                        nc.scalar.activation(kT[:, te, ts(c, N512)], ps,
                                             AF.Identity, bias=bqk[:, 1, te:te + 1])
                    else:
                        nc.vector.tensor_scalar_add(kT[:, te, ts(c, N512)], ps,
                                                    bqk[:, 1, te:te + 1])
                kb_v = kb_in_av if te < 4 else kb_in_bv
                nc.sync.dma_start(kb_v[:, te % 4, :], kT[:, te, :])
                if te == 0:
                    nc.sync.dma_start(
                        wv_sb, wv.ap().rearrange("(t p) e -> p t e", p=P))
                elif te == 3:
                    nc.sync.dma_start(
                        wq_sb, wq.ap().rearrange("(t p) e -> p t e", p=P))
                elif te == DT - 1:
                    pass
                if te == 3:
                    nc.gpsimd.collective_compute(
                        "AllGather", mybir.AluOpType.bypass,
                        replica_groups=groups,
                        ins=[kb_in_a[:].opt()], outs=[kb_out_a[:].opt()])
            nc.gpsimd.collective_compute(
                "AllGather", mybir.AluOpType.bypass, replica_groups=groups,
                ins=[kb_in_b[:].opt()], outs=[kb_out_b[:].opt()])

            # vo : lhsT = xTq-slice, rhs = Wv -> own half of v
            vb_in_v = vb_in[:].rearrange("(t p) d -> p t d", p=P)
            for tsq in range(SQT):
                for dc in range(D // N512):
                    ps = psum.tile([P, N512], f32, tag="mm")
                    for u in range(KP):
                        nc.tensor.matmul(ps, xTq[:, 2 * u:2 * u + 2, ts(tsq, P)],
                                         wv_sb[:, 2 * u:2 * u + 2, ts(dc, N512)],
                                         start=(u == 0), stop=(u == KP - 1),
                                         perf_mode=DR)
                    nc.vector.tensor_add(v[:, tsq, ts(dc, N512)], ps,
                                         bv_sb[:, ts(dc, N512)])
                nc.sync.dma_start(vb_in_v[:, tsq, :], v[:, tsq, :])
                if tsq == 1:
                    nc.sync.dma_start(
                        wo_sb, wo.ap().rearrange("(t p) e -> p t e", p=P))
            nc.gpsimd.collective_compute(
                "AllGather", mybir.AluOpType.bypass, replica_groups=groups,
                ins=[vb_in[:].opt()], outs=[vb_out[:].opt()])

            # peer halves: gather the peer's rows of the AllGather outputs
            # straight into the peer tiles — idx data is per-core
            nc.gpsimd.dma_gather(kTp[:, 0:4, :],
                                 kb_out_a[:].rearrange("r d s -> (r d) s"),
                                 gidxk[:, :], S // 4, S // 4, SQ)
            nc.gpsimd.dma_gather(kTp[:, 4:8, :],
                                 kb_out_b[:].rearrange("r d s -> (r d) s"),
                                 gidxk[:, :], S // 4, S // 4, SQ)
            nc.gpsimd.dma_gather(vp[:, :, :],
                                 vb_out[:].rearrange("r s d -> (r s) d"),
                                 gidxv[:, :], S // 2, S // 2, D)

            # qT : lhsT = Wq-slice, rhs = xTq
            for te in range(DT):
                for c in range(SQ // N512):
                    ps = psum.tile([P, N512], f32, tag="mm")
                    for u in range(KP):
                        nc.tensor.matmul(ps, wq_sb[:, 2 * u:2 * u + 2, ts(te, P)],
                                         xTq[:, 2 * u:2 * u + 2, ts(c, N512)],
                                         start=(u == 0), stop=(u == KP - 1),
                                         perf_mode=DR)
                    if c == 0:
                        nc.scalar.activation(qT[:, te, ts(c, N512)], ps,
                                             AF.Identity, bias=bqk[:, 0, te:te + 1])
                    else:
                        nc.vector.tensor_scalar_add(qT[:, te, ts(c, N512)], ps,
                                                    bqk[:, 0, te:te + 1])

        # ---------------- phase 2: attention + out-proj ----------------
        with ExitStack() as ph2:
            epool = ph2.enter_context(tc.tile_pool(name="ep", bufs=2))
            work = ph2.enter_context(tc.tile_pool(name="wk2", bufs=2))
            opool = ph2.enter_context(tc.tile_pool(name="op2", bufs=2))
            rspool = ph2.enter_context(tc.tile_pool(name="rs2", bufs=2))
            rpool = ph2.enter_context(tc.tile_pool(name="rp", bufs=8))
            dpool = ph2.enter_context(
                tc.tile_pool(name="dram2", bufs=2, space="DRAM"))

            # scores in own-half / peer-half blocks; each chunk's softmax
            # denominator + reciprocal round-trip hides behind later blocks
            expTs, rSbs = [], []
            for c in range(SQ // N512):          # sq chunks of 512
                expT = epool.tile([P, ST, N512], f8, tag="expT",
                                  name=f"expT{c}")
                expTs.append(expT)

            def sc_block(c, tk_lo, tk_hi):
                expT = expTs[c]
                for tk in range(tk_lo, tk_hi):
                    ksrc = kT if tk < SQT else kTp
                    ps = psum.tile([P, N512], f32, tag="mm")
                    for u in range(KP):
                        nc.tensor.matmul(ps,
                                         ksrc[:, 2 * u:2 * u + 2,
                                              ts(tk % SQT, P)],
                                         qT[:, 2 * u:2 * u + 2, ts(c, N512)],
                                         start=(u == 0), stop=(u == KP - 1),
                                         perf_mode=DR)
                    nc.scalar.activation(expT[:, tk, :], ps, AF.Exp,
                                         scale=EXP_SCALE, bias=ebias)

            def s_block(c):
                expT = expTs[c]
                psS = psum_s.tile([1, N512], f32, tag="S")
                for tk in range(ST // 2):
                    nc.tensor.matmul(psS, ones[:, :, :1],
                                     expT[:, 2 * tk:2 * tk + 2, :],
                                     start=(tk == 0), stop=(tk == ST // 2 - 1),
                                     perf_mode=DR)
                rS_row = rspool.tile([1, N512], f32, tag="rS_row",
                                     name=f"rS{c}")
                nc.vector.reciprocal(rS_row, psS)   # = 32 / Sigma exp
                # broadcast 1/S across partitions via DRAM (stride-0 DMA)
                rs_dram = dpool.tile([N512], f32, tag="rs_dram")
                nc.sync.dma_start(
                    rs_dram[:].rearrange("(o s) -> o s", o=1), rS_row)
                rSb = rspool.tile([P, N512], f32, tag="rSb", name=f"rSb{c}")
                nc.scalar.dma_start(rSb, rs_dram[:].partition_broadcast(P))
                rSbs.append(rSb)

            sc_block(0, 0, SQT)        # own keys: no collective dependency
            sc_block(1, 0, SQT)        # more own-key work to hide the wire
            sc_block(0, SQT, ST)       # peer keys: needs AG1 + gathers
            s_block(0)
            sc_block(1, SQT, ST)
            s_block(1)

            # residual prefetch AFTER the exchange window so its HBM reads
            # don't fight the AllGather wire + gathers (bf16: half traffic)
            res_sbs = []
            for sl8 in range(SQT):
                res_sb = rpool.tile([P, D], bf16, tag="res", name=f"res{sl8}")
                nc.sync.dma_start(res_sb, resid.ap()[sl8 * P:(sl8 + 1) * P, :])
                res_sbs.append(res_sb)

            for c in range(SQ // N512):
                expT, rSb = expTs[c], rSbs[c]
                # mixedUT[d, sq] = (v^T-stationary @ expT) / S  (normalized on
                # the psum->fp8 convert; unnormalized would overflow e4m3)
                mixUT = work.tile([P, DT, N512], f8, tag="mixUT",
                                  name=f"mixUT{c}")
                for dsl in range(DT):
                    ps = psum.tile([P, N512], f32, tag="mm")
                    for tk in range(ST // 2):
                        vsrc = v if tk < SQT // 2 else vp
                        nc.tensor.matmul(ps,
                                         vsrc[:, (2 * tk) % SQT:
                                              (2 * tk) % SQT + 2, ts(dsl, P)],
                                         expT[:, 2 * tk:2 * tk + 2, :],
                                         start=(tk == 0), stop=(tk == ST // 2 - 1),
                                         perf_mode=DR)
                    nc.vector.tensor_mul(mixUT[:, dsl, :], ps, rSb)

                for sl in range(4):
                    row = (c * 4 + sl) * P
                    res_sb = res_sbs[c * 4 + sl]
                    out_sb = opool.tile([P, D], f32, tag="osb")
                    for ec in range(D // N512):
                        ps = psum.tile([P, N512], f32, tag="mm")
                        for u in range(KP):
                            nc.tensor.matmul(
                                ps, mixUT[:, 2 * u:2 * u + 2, ts(sl, P)],
                                wo_sb[:, 2 * u:2 * u + 2, ts(ec, N512)],
                                start=(u == 0), stop=(u == KP - 1),
                                perf_mode=DR)
                        # out = psum / 1024 + (residual + bo), one fused op
                        nc.vector.scalar_tensor_tensor(
                            out_sb[:, ts(ec, N512)], ps, OUT_DESCALE,
                            res_sb[:, ts(ec, N512)], ALU.mult, ALU.add)
                    nc.sync.dma_start(out_d.ap()[row:row + P, :], out_sb)

    nc.compile()
    return nc


def _get_program():
    if "nc" not in _COMPILED:
        _COMPILED["nc"] = _build_program()
    return _COMPILED["nc"]


def make_in_maps(tokens, Wq, bq, Wk, bk, Wv, bv, Wo, bo):
    tokens = np.asarray(tokens, dtype=np.float32)
    bf = ml_dtypes.bfloat16
    f8 = ml_dtypes.float8_e4m3
    wq_b = np.ascontiguousarray((np.asarray(Wq, np.float32) * WSCALE).astype(f8))
    wk_b = np.ascontiguousarray((np.asarray(Wk, np.float32) * WSCALE).astype(f8))
    wv_b = np.ascontiguousarray((np.asarray(Wv, np.float32) * WSCALE).astype(f8))
    wo_b = np.ascontiguousarray((np.asarray(Wo, np.float32) * WSCALE).astype(f8))
    bq = np.asarray(bq, np.float32) * WSCALE
    bk = np.asarray(bk, np.float32) * WSCALE
    # center v by c ~ E_k[v] so the fp8 mixUT quantizes the small AC part;
    # softmax weights sum to 1, so out = (mixed-c)@Wo + (c@Wo + bo) + resid.
    wv32 = np.asarray(Wv, np.float32)
    cvec = GELU_MEAN * wv32.sum(axis=0) + np.asarray(bv, np.float32)
    bv = (np.asarray(bv, np.float32) - cvec) * WSCALE
    bo_eff = (np.asarray(bo, np.float32)
              + cvec @ np.asarray(Wo, np.float32)).astype(np.float32)

    pp, mm = np.meshgrid(np.arange(P), np.arange(S // 32), indexing="ij")
    base_v = (mm * 16 + (pp % 16)).astype(np.int16)     # j = m*16 + lane
    base_k = base_v[:, :S // 64]

    in_maps = []
    for c in range(NCORES):
        b, h = divmod(c, 2)
        q_rows = tokens[b, h * SQ:(h + 1) * SQ]
        in_maps.append({
            "tokTq": np.ascontiguousarray(q_rows.T.astype(bf)),  # [D, SQ]
            "resid": np.ascontiguousarray((q_rows + bo_eff).astype(bf)),
            "wq": wq_b, "wk": wk_b, "wv": wv_b, "wo": wo_b,
            "bq": bq, "bk": bk, "bv": bv,
            "gidxk": np.ascontiguousarray(base_k + np.int16((1 - h) * (SQ // 2))),
            "gidxv": np.ascontiguousarray(base_v + np.int16((1 - h) * SQ)),
        })
    return in_maps


def gather_out(results):
    out = np.empty((B, S, D), np.float32)
    for c in range(NCORES):
        b, h = divmod(c, 2)
        out[b, h * SQ:(h + 1) * SQ] = results[c]["out"]
    return out


def kernel(tokens, Wq, bq, Wk, bk, Wv, bv, Wo, bo):
    from concourse.bass_utils import run_bass_kernel_spmd

    in_maps = make_in_maps(tokens, Wq, bq, Wk, bk, Wv, bv, Wo, bo)
    nc = _get_program()
    res = run_bass_kernel_spmd(nc, in_maps, core_ids=list(range(NCORES)),
                               trace=False)
    return gather_out(res.results)


# revision 18
# speedup vs baseline: 1.2381x; 1.0049x over previous
"""Trainium2 Bass kernel for a single-head attention block (B=4, S=2048, D=1024).

reference:
    x = gelu(tokens); q,k,v = x@W{q,k,v} + b; scores = q@k^T/sqrt(D)
    out = softmax(scores)@v @ Wo + bo + tokens

Sharding: 8 cores = 4 batches x 2 query-halves. Core c=2b+h handles batch b and
query rows [h*1024, (h+1)*1024). Each core computes q/k/v projections for its
own rows only; K^T and V halves are exchanged pairwise via two AllGathers. The
fp32 residual path dominates the output magnitude, so the whole matmul pipeline
runs in fp8-e4m3 with DoubleRow perf mode (K=256 per matmul).

KEY LAYOUT TRICK (v4): softmax is permutation-invariant over the key axis as
long as k and v use the SAME order, so each core keeps its OWN key/value rows
in tiles [0, SQ) of kT/v and the PEER's rows in [SQ, 2*SQ). The projection
evictions write straight into the own half (no copy), and the peer half is
pulled from the AllGather output with a dma_gather whose int16 row indices are
HOST-PROVIDED per-core data (peer slot = 1-h) — the program stays SPMD-uniform
while the own-half scores run with no dependency on the collective at all.

Scales: weights are pre-scaled x32 on the host (sigma~1 in fp8), so stored
q,k,v are 32x true scale. scores_psum = 32768*scores_true -> exp uses
scale=2^-15, bias=-5ln2, giving expT = exp(scores)/32 in fp8. Softmax
denominators via a ones-stationary matmul; rS_row = 1/Sigma exp is broadcast
across partitions via a DRAM round-trip (hidden behind the other chunk's
scores). The mixed psum is normalized on the psum->fp8 DVE convert (v is
centered host-side so the fp8 mixUT quantizes the small AC part). The out-proj
psum is 32*(mixed@Wo)*32, folded by 1/1024 on the fused
(psum*c + residual) DVE op; bo and the centering correction are pre-added into
the residual on the host.

Schedule: PSUM evictions alternate ACT/DVE; PE order is
  warmup | kTo -> AG1 | vo -> AG2 | qT | sc0-own sc0-peer S0 | sc1-own
  sc1-peer S1 | mix0 out0 | mix1 out1
so the AllGather wire+gather latency hides behind qT+own-half scores, and each
chunk's softmax reciprocal round-trip hides behind the other chunk's work.
"""

import math

import numpy as np
import ml_dtypes

B, S, D = 4, 2048, 1024
NCORES = 8
SQ = S // 2          # query rows per core
P = 128
DT = 8               # d / 128
KP = DT // 2         # K-pair count for DoubleRow (K=256 each)
ST = S // P          # 16 seq tiles
SQT = SQ // P        # 8
N512 = 512
WARMUP_MMS = 20
WSCALE = 32.0        # host-side weight/bias scale
EXP_BIAS = -5.0 * math.log(2.0)   # expT = exp(scores)/32
EXP_SCALE = 1.0 / 32768.0         # scores_psum = 32768 * scores_true
OUT_DESCALE = 1.0 / 1024.0
GELU_MEAN = 0.3989422804014327    # E[gelu(z)], z ~ N(0,1)

_COMPILED = {}


def _build_program():
    from contextlib import ExitStack

    import concourse.bass as bass
    import concourse.tile as tile
    from concourse import bacc, mybir

    f32 = mybir.dt.float32
    bf16 = mybir.dt.bfloat16
    f8 = mybir.dt.float8e4
    i16 = mybir.dt.int16
    AF = mybir.ActivationFunctionType
    ALU = mybir.AluOpType
    DR = mybir.MatmulPerfMode.DoubleRow

    nc = bacc.Bacc("TRN2", target_bir_lowering=False, debug=False,
                   num_devices=NCORES)

    tokTq = nc.dram_tensor("tokTq", [D, SQ], bf16, kind="ExternalInput")
    resid = nc.dram_tensor("resid", [SQ, D], bf16, kind="ExternalInput")
    wq = nc.dram_tensor("wq", [D, D], f8, kind="ExternalInput")
    wk = nc.dram_tensor("wk", [D, D], f8, kind="ExternalInput")
    wv = nc.dram_tensor("wv", [D, D], f8, kind="ExternalInput")
    wo = nc.dram_tensor("wo", [D, D], f8, kind="ExternalInput")
    bq_d = nc.dram_tensor("bq", [D], f32, kind="ExternalInput")   # x32
    bk_d = nc.dram_tensor("bk", [D], f32, kind="ExternalInput")   # x32
    bv_d = nc.dram_tensor("bv", [D], f32, kind="ExternalInput")   # x32
    gidxk_d = nc.dram_tensor("gidxk", [P, S // 64], i16, kind="ExternalInput")
    out_d = nc.dram_tensor("out", [SQ, D], f32, kind="ExternalOutput")

    ts = bass.ts
    groups = [[2 * i, 2 * i + 1] for i in range(NCORES // 2)]

    with tile.TileContext(nc) as tc, ExitStack() as ctx:
        pers = ctx.enter_context(tc.tile_pool(name="pers", bufs=1))
        kT = pers.tile([P, DT, SQ], f8, tag="kT")     # own keys
        kTp = pers.tile([P, DT, SQ], f8, tag="kTp")   # peer keys
        qT = pers.tile([P, DT, SQ], f8, tag="qT")
        v = pers.tile([P, SQT, D], f8, tag="v")       # own values
        vp = pers.tile([P, SQT, D], f8, tag="vp")     # peer values
        ones = pers.tile([P, 2, 16], f8, tag="ones")
        bqk = pers.tile([P, 2, DT], f32, tag="bqk")  # [:,0,:]=32bq [:,1,:]=32bk
        ebias = pers.tile([P, 1], f32, tag="ebias")
        wscr = pers.tile([P, N512], bf16, tag="wscr")
        wsink = pers.tile([P, P], f32, tag="wsink")
        wo_sb = pers.tile([P, DT, D], f8, tag="wo")
        gidxk = pers.tile([P, S // 64], i16, tag="gidxk")

        dram = ctx.enter_context(tc.tile_pool(name="dram", bufs=1, space="DRAM"))
        bar_in = dram.tile([64], f8, tag="bar_in")
        bar_out = dram.tile([2, 64], f8, tag="bar_out")
        kb_in_a = dram.tile([D // 2, SQ], f8, tag="kb_in_a")
        kb_in_b = dram.tile([D // 2, SQ], f8, tag="kb_in_b")
        kb_out_a = dram.tile([2, D // 2, SQ], f8, tag="kb_out_a")
        kb_out_b = dram.tile([2, D // 2, SQ], f8, tag="kb_out_b")
        vb_in_a = dram.tile([SQ // 2, D], f8, tag="vb_in_a")
        vb_in_b = dram.tile([SQ // 2, D], f8, tag="vb_in_b")
        vb_out_a = dram.tile([2, SQ // 2, D], f8, tag="vb_out_a")
        vb_out_b = dram.tile([2, SQ // 2, D], f8, tag="vb_out_b")

        psum = ctx.enter_context(tc.tile_pool(name="psum", bufs=5, space="PSUM"))
        psum_s = ctx.enter_context(tc.tile_pool(name="psum_s", bufs=2, space="PSUM"))

        # --- PE warm-up: dense trivial matmuls so HAM hits K=8/8 and PE is
        # busy while the gelu+DMA head runs.
        nc.vector.memset(wscr, 0.0)
        wps = psum.tile([P, N512], f32, tag="mm")
        for i in range(WARMUP_MMS):
            nc.tensor.matmul(wps, wscr[:, :P], wscr, start=(i == 0),
                             stop=(i == WARMUP_MMS - 1))
        nc.vector.tensor_copy(wsink, wps[:, :P])

        nc.vector.memset(ones, 1.0)
        nc.vector.memset(ebias, EXP_BIAS)
        nc.scalar.dma_start(bqk[:, 0, :], bq_d.ap().rearrange("(t p) -> p t", p=P))
        nc.scalar.dma_start(bqk[:, 1, :], bk_d.ap().rearrange("(t p) -> p t", p=P))

        # ---------------- phase 1: gelu + projections + kT/v exchange -------
        with ExitStack() as ph1:
            p1 = ph1.enter_context(tc.tile_pool(name="p1", bufs=1))
            xTq = p1.tile([P, DT, SQ], f8, tag="xTq")
            wk_sb = p1.tile([P, DT, D], f8, tag="wk")
            wq_sb = p1.tile([P, DT, D], f8, tag="wq")
            wv_sb = p1.tile([P, DT, D], f8, tag="wv")
            bv_sb = p1.tile([P, D], f32, tag="bv")
            stag = ph1.enter_context(tc.tile_pool(name="stag", bufs=4))

            # Head is HBM-bound: load ONLY what the gelu needs now (tokens +
            # Wk); Wv/Wq/Wo triggers are interleaved into the staging loops
            # below so their 3MB doesn't steal HBM bandwidth from the tokens.
            nc.gpsimd.dma_start(wk_sb,
                                wk.ap().rearrange("(t p) e -> p t e", p=P))
            nc.gpsimd.dma_start(
                bv_sb, bass.AP(tensor=bv_d, offset=0, ap=[[0, P], [1, D]]))
            nc.gpsimd.dma_start(gidxk, gidxk_d.ap())
            # tiny pair-barrier during the gelu head: absorbs the cross-core
            # launch skew so the first real AllGather's wire starts promptly
            nc.gpsimd.collective_compute(
                "AllGather", mybir.AluOpType.bypass, replica_groups=groups,
                ins=[bar_in[:].opt()], outs=[bar_out[:].opt()])
            # tokens in 4 pair-tiles; gelu per pair so each ACT op unlocks a
            # full DoubleRow K-pair for the projection matmuls
            for g in range(DT // 2):
                stq = stag.tile([P, 2, SQ], bf16, tag="tok", name=f"stq{g}")
                nc.sync.dma_start(
                    stq, tokTq.ap()[2 * g * P:(2 * g + 2) * P, :].rearrange(
                        "(t p) s -> p t s", p=P))
                nc.scalar.activation(xTq[:, 2 * g:2 * g + 2, :], stq, AF.Gelu)

            # kTo: lhsT = Wk-slice, rhs = xTq -> write own half of kT
            # directly; stream each te row-block to DRAM as its converts
            # land. The exchange is split into two half-AllGathers so the
            # first fires as soon as te 0-3 are staged.
            kb_in_av = kb_in_a[:].rearrange("(t p) s -> p t s", p=P)
            kb_in_bv = kb_in_b[:].rearrange("(t p) s -> p t s", p=P)
            for te in range(DT):
                for c in range(SQ // N512):
                    ps = psum.tile([P, N512], f32, tag="mm")
                    for u in range(KP):
                        nc.tensor.matmul(ps, wk_sb[:, 2 * u:2 * u + 2, ts(te, P)],
                                         xTq[:, 2 * u:2 * u + 2, ts(c, N512)],
                                         start=(u == 0), stop=(u == KP - 1),
                                         perf_mode=DR)
                    if c == 0:
                        nc.scalar.activation(kT[:, te, ts(c, N512)], ps,
                                             AF.Identity, bias=bqk[:, 1, te:te + 1])
                    else:
                        nc.vector.tensor_scalar_add(kT[:, te, ts(c, N512)], ps,
                                                    bqk[:, 1, te:te + 1])
                kb_v = kb_in_av if te < 4 else kb_in_bv
                nc.sync.dma_start(kb_v[:, te % 4, :], kT[:, te, :])
                if te == 0:
                    nc.sync.dma_start(
                        wv_sb, wv.ap().rearrange("(t p) e -> p t e", p=P))
                elif te == 3:
                    nc.sync.dma_start(
                        wq_sb, wq.ap().rearrange("(t p) e -> p t e", p=P))
                elif te == DT - 1:
                    pass
                if te == 3:
                    nc.gpsimd.collective_compute(
                        "AllGather", mybir.AluOpType.bypass,
                        replica_groups=groups,
                        ins=[kb_in_a[:].opt()], outs=[kb_out_a[:].opt()])
            nc.gpsimd.collective_compute(
                "AllGather", mybir.AluOpType.bypass, replica_groups=groups,
                ins=[kb_in_b[:].opt()], outs=[kb_out_b[:].opt()])

            # vo : lhsT = xTq-slice, rhs = Wv -> own half of v; exchange
            # split in two half-AllGathers like the keys
            vb_in_av = vb_in_a[:].rearrange("(t p) d -> p t d", p=P)
            vb_in_bv = vb_in_b[:].rearrange("(t p) d -> p t d", p=P)
            for tsq in range(SQT):
                for dc in range(D // N512):
                    ps = psum.tile([P, N512], f32, tag="mm")
                    for u in range(KP):
                        nc.tensor.matmul(ps, xTq[:, 2 * u:2 * u + 2, ts(tsq, P)],
                                         wv_sb[:, 2 * u:2 * u + 2, ts(dc, N512)],
                                         start=(u == 0), stop=(u == KP - 1),
                                         perf_mode=DR)
                    nc.vector.tensor_add(v[:, tsq, ts(dc, N512)], ps,
                                         bv_sb[:, ts(dc, N512)])
                vb_v = vb_in_av if tsq < 4 else vb_in_bv
                nc.sync.dma_start(vb_v[:, tsq % 4, :], v[:, tsq, :])
                if tsq == 1:
                    nc.sync.dma_start(
                        wo_sb, wo.ap().rearrange("(t p) e -> p t e", p=P))
                if tsq == 3:
                    nc.gpsimd.collective_compute(
                        "AllGather", mybir.AluOpType.bypass,
                        replica_groups=groups,
                        ins=[vb_in_a[:].opt()], outs=[vb_out_a[:].opt()])
            nc.gpsimd.collective_compute(
                "AllGather", mybir.AluOpType.bypass, replica_groups=groups,
                ins=[vb_in_b[:].opt()], outs=[vb_out_b[:].opt()])

            # peer halves: gather the peer's rows of the AllGather outputs
            # straight into the peer tiles — idx data is per-core
            nc.gpsimd.dma_gather(kTp[:, 0:4, :],
                                 kb_out_a[:].rearrange("r d s -> (r d) s"),
                                 gidxk[:, :], S // 4, S // 4, SQ)
            nc.gpsimd.dma_gather(kTp[:, 4:8, :],
                                 kb_out_b[:].rearrange("r d s -> (r d) s"),
                                 gidxk[:, :], S // 4, S // 4, SQ)
            nc.gpsimd.dma_gather(vp[:, 0:4, :],
                                 vb_out_a[:].rearrange("r s d -> (r s) d"),
                                 gidxk[:, :], S // 4, S // 4, D)
            nc.gpsimd.dma_gather(vp[:, 4:8, :],
                                 vb_out_b[:].rearrange("r s d -> (r s) d"),
                                 gidxk[:, :], S // 4, S // 4, D)

            # qT : lhsT = Wq-slice, rhs = xTq
            for te in range(DT):
                for c in range(SQ // N512):
                    ps = psum.tile([P, N512], f32, tag="mm")
                    for u in range(KP):
                        nc.tensor.matmul(ps, wq_sb[:, 2 * u:2 * u + 2, ts(te, P)],
                                         xTq[:, 2 * u:2 * u + 2, ts(c, N512)],
                                         start=(u == 0), stop=(u == KP - 1),
                                         perf_mode=DR)
                    if c == 0:
                        nc.scalar.activation(qT[:, te, ts(c, N512)], ps,
                                             AF.Identity, bias=bqk[:, 0, te:te + 1])
                    else:
                        nc.vector.tensor_scalar_add(qT[:, te, ts(c, N512)], ps,
                                                    bqk[:, 0, te:te + 1])

        # ---------------- phase 2: attention + out-proj ----------------
        with ExitStack() as ph2:
            epool = ph2.enter_context(tc.tile_pool(name="ep", bufs=2))
            work = ph2.enter_context(tc.tile_pool(name="wk2", bufs=2))
            opool = ph2.enter_context(tc.tile_pool(name="op2", bufs=2))
            rspool = ph2.enter_context(tc.tile_pool(name="rs2", bufs=2))
            rpool = ph2.enter_context(tc.tile_pool(name="rp", bufs=8))
            dpool = ph2.enter_context(
                tc.tile_pool(name="dram2", bufs=2, space="DRAM"))

            # scores in own-half / peer-half blocks; each chunk's softmax
            # denominator + reciprocal round-trip hides behind later blocks
            expTs, rSbs = [], []
            for c in range(SQ // N512):          # sq chunks of 512
                expT = epool.tile([P, ST, N512], f8, tag="expT",
                                  name=f"expT{c}")
                expTs.append(expT)

            def sc_block(c, tk_lo, tk_hi):
                expT = expTs[c]
                for tk in range(tk_lo, tk_hi):
                    ksrc = kT if tk < SQT else kTp
                    ps = psum.tile([P, N512], f32, tag="mm")
                    for u in range(KP):
                        nc.tensor.matmul(ps,
                                         ksrc[:, 2 * u:2 * u + 2,
                                              ts(tk % SQT, P)],
                                         qT[:, 2 * u:2 * u + 2, ts(c, N512)],
                                         start=(u == 0), stop=(u == KP - 1),
                                         perf_mode=DR)
                    nc.scalar.activation(expT[:, tk, :], ps, AF.Exp,
                                         scale=EXP_SCALE, bias=ebias)

            def s_block(c):
                expT = expTs[c]
                psS = psum_s.tile([1, N512], f32, tag="S")
                for tk in range(ST // 2):
                    nc.tensor.matmul(psS, ones[:, :, :1],
                                     expT[:, 2 * tk:2 * tk + 2, :],
                                     start=(tk == 0), stop=(tk == ST // 2 - 1),
                                     perf_mode=DR)
                rS_row = rspool.tile([1, N512], f32, tag="rS_row",
                                     name=f"rS{c}")
                nc.vector.reciprocal(rS_row, psS)   # = 32 / Sigma exp
                # broadcast 1/S across partitions via DRAM (stride-0 DMA)
                rs_dram = dpool.tile([N512], f32, tag="rs_dram")
                nc.sync.dma_start(
                    rs_dram[:].rearrange("(o s) -> o s", o=1), rS_row)
                rSb = rspool.tile([P, N512], f32, tag="rSb", name=f"rSb{c}")
                nc.scalar.dma_start(rSb, rs_dram[:].partition_broadcast(P))
                rSbs.append(rSb)

            sc_block(0, 0, SQT)        # own keys: no collective dependency
            sc_block(1, 0, SQT)        # more own-key work to hide the wire
            sc_block(0, SQT, ST)       # peer keys: needs AG1 + gathers
            s_block(0)
            sc_block(1, SQT, ST)
            s_block(1)

            # residual prefetch AFTER the exchange window so its HBM reads
            # don't fight the AllGather wire + gathers (bf16: half traffic)
            res_sbs = []
            for sl8 in range(SQT):
                res_sb = rpool.tile([P, D], bf16, tag="res", name=f"res{sl8}")
                nc.sync.dma_start(res_sb, resid.ap()[sl8 * P:(sl8 + 1) * P, :])
                res_sbs.append(res_sb)

            for c in range(SQ // N512):
                expT, rSb = expTs[c], rSbs[c]
                # mixedUT[d, sq] = (v^T-stationary @ expT) / S  (normalized on
                # the psum->fp8 convert; unnormalized would overflow e4m3)
                mixUT = work.tile([P, DT, N512], f8, tag="mixUT",
                                  name=f"mixUT{c}")
                for dsl in range(DT):
                    ps = psum.tile([P, N512], f32, tag="mm")
                    for tk in range(ST // 2):
                        vsrc = v if tk < SQT // 2 else vp
                        nc.tensor.matmul(ps,
                                         vsrc[:, (2 * tk) % SQT:
                                              (2 * tk) % SQT + 2, ts(dsl, P)],
                                         expT[:, 2 * tk:2 * tk + 2, :],
                                         start=(tk == 0), stop=(tk == ST // 2 - 1),
                                         perf_mode=DR)
                    nc.vector.tensor_mul(mixUT[:, dsl, :], ps, rSb)

                for sl in range(4):
                    row = (c * 4 + sl) * P
                    res_sb = res_sbs[c * 4 + sl]
                    out_sb = opool.tile([P, D], f32, tag="osb")
                    osc = opool.tile([P, N512], f32, tag="osc")
                    for ec in range(D // N512):
                        ps = psum.tile([P, N512], f32, tag="mm")
                        for u in range(KP):
                            nc.tensor.matmul(
                                ps, mixUT[:, 2 * u:2 * u + 2, ts(sl, P)],
                                wo_sb[:, 2 * u:2 * u + 2, ts(ec, N512)],
                                start=(u == 0), stop=(u == KP - 1),
                                perf_mode=DR)
                        # out = psum / 1024 + (residual + bo); alternate the
                        # evict between DVE (fused) and ACT+GpSimd
                        if ec == 0:
                            nc.vector.scalar_tensor_tensor(
                                out_sb[:, ts(ec, N512)], ps, OUT_DESCALE,
                                res_sb[:, ts(ec, N512)], ALU.mult, ALU.add)
                        else:
                            nc.scalar.activation(osc, ps, AF.Identity,
                                                 scale=OUT_DESCALE)
                            nc.gpsimd.tensor_add(out_sb[:, ts(ec, N512)], osc,
                                                 res_sb[:, ts(ec, N512)])
                    nc.sync.dma_start(out_d.ap()[row:row + P, :], out_sb)

    nc.compile()
    return nc


def _get_program():
    if "nc" not in _COMPILED:
        _COMPILED["nc"] = _build_program()
    return _COMPILED["nc"]


def make_in_maps(tokens, Wq, bq, Wk, bk, Wv, bv, Wo, bo):
    tokens = np.asarray(tokens, dtype=np.float32)
    bf = ml_dtypes.bfloat16
    f8 = ml_dtypes.float8_e4m3
    wq_b = np.ascontiguousarray((np.asarray(Wq, np.float32) * WSCALE).astype(f8))
    wk_b = np.ascontiguousarray((np.asarray(Wk, np.float32) * WSCALE).astype(f8))
    wv_b = np.ascontiguousarray((np.asarray(Wv, np.float32) * WSCALE).astype(f8))
    wo_b = np.ascontiguousarray((np.asarray(Wo, np.float32) * WSCALE).astype(f8))
    bq = np.asarray(bq, np.float32) * WSCALE
    bk = np.asarray(bk, np.float32) * WSCALE
    # center v by c ~ E_k[v] so the fp8 mixUT quantizes the small AC part;
    # softmax weights sum to 1, so out = (mixed-c)@Wo + (c@Wo + bo) + resid.
    wv32 = np.asarray(Wv, np.float32)
    cvec = GELU_MEAN * wv32.sum(axis=0) + np.asarray(bv, np.float32)
    bv = (np.asarray(bv, np.float32) - cvec) * WSCALE
    bo_eff = (np.asarray(bo, np.float32)
              + cvec @ np.asarray(Wo, np.float32)).astype(np.float32)

    pp, mm = np.meshgrid(np.arange(P), np.arange(S // 64), indexing="ij")
    base_k = (mm * 16 + (pp % 16)).astype(np.int16)     # j = m*16 + lane

    in_maps = []
    for c in range(NCORES):
        b, h = divmod(c, 2)
        q_rows = tokens[b, h * SQ:(h + 1) * SQ]
        in_maps.append({
            "tokTq": np.ascontiguousarray(q_rows.T.astype(bf)),  # [D, SQ]
            "resid": np.ascontiguousarray((q_rows + bo_eff).astype(bf)),
            "wq": wq_b, "wk": wk_b, "wv": wv_b, "wo": wo_b,
            "bq": bq, "bk": bk, "bv": bv,
            "gidxk": np.ascontiguousarray(base_k + np.int16((1 - h) * (SQ // 2))),
        })
    return in_maps


def gather_out(results):
    out = np.empty((B, S, D), np.float32)
    for c in range(NCORES):
        b, h = divmod(c, 2)
        out[b, h * SQ:(h + 1) * SQ] = results[c]["out"]
    return out


def kernel(tokens, Wq, bq, Wk, bk, Wv, bv, Wo, bo):
    from concourse.bass_utils import run_bass_kernel_spmd

    in_maps = make_in_maps(tokens, Wq, bq, Wk, bk, Wv, bv, Wo, bo)
    nc = _get_program()
    res = run_bass_kernel_spmd(nc, in_maps, core_ids=list(range(NCORES)),
                               trace=False)
    return gather_out(res.results)


# revision 19
# speedup vs baseline: 1.2797x; 1.0336x over previous
"""Trainium2 Bass kernel for a single-head attention block (B=4, S=2048, D=1024).

reference:
    x = gelu(tokens); q,k,v = x@W{q,k,v} + b; scores = q@k^T/sqrt(D)
    out = softmax(scores)@v @ Wo + bo + tokens

Sharding: 8 cores = 4 batches x 2 query-halves. Core c=2b+h handles batch b and
query rows [h*1024, (h+1)*1024). Each core computes q/k/v projections for its
own rows only; K^T and V halves are exchanged pairwise via two AllGathers. The
fp32 residual path dominates the output magnitude, so the whole matmul pipeline
runs in fp8-e4m3 with DoubleRow perf mode (K=256 per matmul).

KEY LAYOUT TRICK (v4): softmax is permutation-invariant over the key axis as
long as k and v use the SAME order, so each core keeps its OWN key/value rows
in tiles [0, SQ) of kT/v and the PEER's rows in [SQ, 2*SQ). The projection
evictions write straight into the own half (no copy), and the peer half is
pulled from the AllGather output with a dma_gather whose int16 row indices are
HOST-PROVIDED per-core data (peer slot = 1-h) — the program stays SPMD-uniform
while the own-half scores run with no dependency on the collective at all.

Scales: weights are pre-scaled x32 on the host (sigma~1 in fp8), so stored
q,k,v are 32x true scale. scores_psum = 32768*scores_true -> exp uses
scale=2^-15, bias=-5ln2, giving expT = exp(scores)/32 in fp8. Softmax
denominators via a ones-stationary matmul; rS_row = 1/Sigma exp is broadcast
across partitions via a DRAM round-trip (hidden behind the other chunk's
scores). The mixed psum is normalized on the psum->fp8 DVE convert (v is
centered host-side so the fp8 mixUT quantizes the small AC part). The out-proj
psum is 32*(mixed@Wo)*32, folded by 1/1024 on the fused
(psum*c + residual) DVE op; bo and the centering correction are pre-added into
the residual on the host.

Schedule: PSUM evictions alternate ACT/DVE; PE order is
  warmup | kTo -> AG1 | vo -> AG2 | qT | sc0-own sc0-peer S0 | sc1-own
  sc1-peer S1 | mix0 out0 | mix1 out1
so the AllGather wire+gather latency hides behind qT+own-half scores, and each
chunk's softmax reciprocal round-trip hides behind the other chunk's work.
"""

import math

import numpy as np
import ml_dtypes

B, S, D = 4, 2048, 1024
NCORES = 8
SQ = S // 2          # query rows per core
P = 128
DT = 8               # d / 128
KP = DT // 2         # K-pair count for DoubleRow (K=256 each)
ST = S // P          # 16 seq tiles
SQT = SQ // P        # 8
N512 = 512
WARMUP_MMS = 28
WSCALE = 32.0        # host-side weight/bias scale
EXP_BIAS = -5.0 * math.log(2.0)   # expT = exp(scores)/32
EXP_SCALE = 1.0 / 32768.0         # scores_psum = 32768 * scores_true
OUT_DESCALE = 1.0 / 1024.0
GELU_MEAN = 0.3989422804014327    # E[gelu(z)], z ~ N(0,1)

_COMPILED = {}


def _build_program():
    from contextlib import ExitStack

    import concourse.bass as bass
    import concourse.tile as tile
    from concourse import bacc, mybir

    f32 = mybir.dt.float32
    bf16 = mybir.dt.bfloat16
    f8 = mybir.dt.float8e4
    i16 = mybir.dt.int16
    AF = mybir.ActivationFunctionType
    ALU = mybir.AluOpType
    DR = mybir.MatmulPerfMode.DoubleRow

    nc = bacc.Bacc("TRN2", target_bir_lowering=False, debug=False,
                   num_devices=NCORES)

    tokTq = nc.dram_tensor("tokTq", [D, SQ], bf16, kind="ExternalInput")
    resid = nc.dram_tensor("resid", [SQ, D], bf16, kind="ExternalInput")
    wq = nc.dram_tensor("wq", [D, D], f8, kind="ExternalInput")
    wk = nc.dram_tensor("wk", [D, D], f8, kind="ExternalInput")
    wv = nc.dram_tensor("wv", [D, D], f8, kind="ExternalInput")
    wo = nc.dram_tensor("wo", [D, D], f8, kind="ExternalInput")
    bq_d = nc.dram_tensor("bq", [D], f32, kind="ExternalInput")   # x32
    bk_d = nc.dram_tensor("bk", [D], f32, kind="ExternalInput")   # x32
    bv_d = nc.dram_tensor("bv", [D], f32, kind="ExternalInput")   # x32
    gidxk_d = nc.dram_tensor("gidxk", [P, S // 64], i16, kind="ExternalInput")
    out_d = nc.dram_tensor("out", [SQ, D], f32, kind="ExternalOutput")

    ts = bass.ts
    groups = [[2 * i, 2 * i + 1] for i in range(NCORES // 2)]

    with tile.TileContext(nc) as tc, ExitStack() as ctx:
        pers = ctx.enter_context(tc.tile_pool(name="pers", bufs=1))
        kT = pers.tile([P, DT, SQ], f8, tag="kT")     # own keys
        kTp = pers.tile([P, DT, SQ], f8, tag="kTp")   # peer keys
        qT = pers.tile([P, DT, SQ], f8, tag="qT")
        v = pers.tile([P, SQT, D], f8, tag="v")       # own values
        vp = pers.tile([P, SQT, D], f8, tag="vp")     # peer values
        ones = pers.tile([P, 2, 16], f8, tag="ones")
        bqk = pers.tile([P, 2, DT], f32, tag="bqk")  # [:,0,:]=32bq [:,1,:]=32bk
        ebias = pers.tile([P, 1], f32, tag="ebias")
        wscr = pers.tile([P, N512], bf16, tag="wscr")
        wsink = pers.tile([P, P], f32, tag="wsink")
        wo_sb = pers.tile([P, DT, D], f8, tag="wo")
        gidxk = pers.tile([P, S // 64], i16, tag="gidxk")

        dram = ctx.enter_context(tc.tile_pool(name="dram", bufs=1, space="DRAM"))
        kb_in_a = dram.tile([D // 2, SQ], f8, tag="kb_in_a")
        kb_in_b = dram.tile([D // 2, SQ], f8, tag="kb_in_b")
        kb_out_a = dram.tile([2, D // 2, SQ], f8, tag="kb_out_a")
        kb_out_b = dram.tile([2, D // 2, SQ], f8, tag="kb_out_b")
        vb_in_a = dram.tile([SQ // 2, D], f8, tag="vb_in_a")
        vb_in_b = dram.tile([SQ // 2, D], f8, tag="vb_in_b")
        vb_out_a = dram.tile([2, SQ // 2, D], f8, tag="vb_out_a")
        vb_out_b = dram.tile([2, SQ // 2, D], f8, tag="vb_out_b")

        psum = ctx.enter_context(tc.tile_pool(name="psum", bufs=6, space="PSUM"))
        psum_s = ctx.enter_context(tc.tile_pool(name="psum_s", bufs=2, space="PSUM"))

        # --- PE warm-up: dense trivial matmuls so HAM hits K=8/8 and PE is
        # busy while the gelu+DMA head runs.
        nc.vector.memset(wscr, 0.0)
        wps = psum.tile([P, N512], f32, tag="mm")
        for i in range(WARMUP_MMS):
            nc.tensor.matmul(wps, wscr[:, :P], wscr, start=(i == 0),
                             stop=(i == WARMUP_MMS - 1))
        nc.vector.tensor_copy(wsink, wps[:, :P])

        nc.vector.memset(ones, 1.0)
        nc.vector.memset(ebias, EXP_BIAS)
        nc.scalar.dma_start(bqk[:, 0, :], bq_d.ap().rearrange("(t p) -> p t", p=P))
        nc.scalar.dma_start(bqk[:, 1, :], bk_d.ap().rearrange("(t p) -> p t", p=P))

        # ---------------- phase 1: gelu + projections + kT/v exchange -------
        with ExitStack() as ph1:
            p1 = ph1.enter_context(tc.tile_pool(name="p1", bufs=1))
            xTq = p1.tile([P, DT, SQ], f8, tag="xTq")
            wk_sb = p1.tile([P, DT, D], f8, tag="wk")
            wq_sb = p1.tile([P, DT, D], f8, tag="wq")
            wv_sb = p1.tile([P, DT, D], f8, tag="wv")
            bv_sb = p1.tile([P, D], f32, tag="bv")
            stag = ph1.enter_context(tc.tile_pool(name="stag", bufs=4))

            # Head is HBM-bound: load ONLY what the gelu needs now (tokens +
            # Wk); Wv/Wq/Wo triggers are interleaved into the staging loops
            # below so their 3MB doesn't steal HBM bandwidth from the tokens.
            nc.gpsimd.dma_start(wk_sb,
                                wk.ap().rearrange("(t p) e -> p t e", p=P))
            nc.gpsimd.dma_start(
                bv_sb, bass.AP(tensor=bv_d, offset=0, ap=[[0, P], [1, D]]))
            nc.gpsimd.dma_start(gidxk, gidxk_d.ap())
            # tokens in 4 pair-tiles; gelu per pair so each ACT op unlocks a
            # full DoubleRow K-pair for the projection matmuls
            for g in range(DT // 2):
                stq = stag.tile([P, 2, SQ], bf16, tag="tok", name=f"stq{g}")
                nc.sync.dma_start(
                    stq, tokTq.ap()[2 * g * P:(2 * g + 2) * P, :].rearrange(
                        "(t p) s -> p t s", p=P))
                nc.scalar.activation(xTq[:, 2 * g:2 * g + 2, :], stq, AF.Gelu)

            # kTo: lhsT = Wk-slice, rhs = xTq -> write own half of kT
            # directly; stream each te row-block to DRAM as its converts
            # land. The exchange is split into two half-AllGathers so the
            # first fires as soon as te 0-3 are staged.
            kb_in_av = kb_in_a[:].rearrange("(t p) s -> p t s", p=P)
            kb_in_bv = kb_in_b[:].rearrange("(t p) s -> p t s", p=P)
            for te in range(DT):
                for c in range(SQ // N512):
                    ps = psum.tile([P, N512], f32, tag="mm")
                    for u in range(KP):
                        nc.tensor.matmul(ps, wk_sb[:, 2 * u:2 * u + 2, ts(te, P)],
                                         xTq[:, 2 * u:2 * u + 2, ts(c, N512)],
                                         start=(u == 0), stop=(u == KP - 1),
                                         perf_mode=DR)
                    if c == 0:
                        nc.scalar.activation(kT[:, te, ts(c, N512)], ps,
                                             AF.Identity, bias=bqk[:, 1, te:te + 1])
                    else:
                        nc.vector.tensor_scalar_add(kT[:, te, ts(c, N512)], ps,
                                                    bqk[:, 1, te:te + 1])
                kb_v = kb_in_av if te < 4 else kb_in_bv
                nc.sync.dma_start(kb_v[:, te % 4, :], kT[:, te, :])
                if te == 0:
                    nc.sync.dma_start(
                        wv_sb, wv.ap().rearrange("(t p) e -> p t e", p=P))
                elif te == 3:
                    nc.sync.dma_start(
                        wq_sb, wq.ap().rearrange("(t p) e -> p t e", p=P))
                elif te == DT - 1:
                    pass
                if te == 3:
                    nc.gpsimd.collective_compute(
                        "AllGather", mybir.AluOpType.bypass,
                        replica_groups=groups,
                        ins=[kb_in_a[:].opt()], outs=[kb_out_a[:].opt()])
            nc.gpsimd.collective_compute(
                "AllGather", mybir.AluOpType.bypass, replica_groups=groups,
                ins=[kb_in_b[:].opt()], outs=[kb_out_b[:].opt()])

            # vo : lhsT = xTq-slice, rhs = Wv -> own half of v; exchange
            # split in two half-AllGathers like the keys
            vb_in_av = vb_in_a[:].rearrange("(t p) d -> p t d", p=P)
            vb_in_bv = vb_in_b[:].rearrange("(t p) d -> p t d", p=P)
            for tsq in range(SQT):
                for dc in range(D // N512):
                    ps = psum.tile([P, N512], f32, tag="mm")
                    for u in range(KP):
                        nc.tensor.matmul(ps, xTq[:, 2 * u:2 * u + 2, ts(tsq, P)],
                                         wv_sb[:, 2 * u:2 * u + 2, ts(dc, N512)],
                                         start=(u == 0), stop=(u == KP - 1),
                                         perf_mode=DR)
                    nc.vector.tensor_add(v[:, tsq, ts(dc, N512)], ps,
                                         bv_sb[:, ts(dc, N512)])
                vb_v = vb_in_av if tsq < 4 else vb_in_bv
                nc.sync.dma_start(vb_v[:, tsq % 4, :], v[:, tsq, :])
                if tsq == 1:
                    nc.sync.dma_start(
                        wo_sb, wo.ap().rearrange("(t p) e -> p t e", p=P))
                if tsq == 3:
                    nc.gpsimd.collective_compute(
                        "AllGather", mybir.AluOpType.bypass,
                        replica_groups=groups,
                        ins=[vb_in_a[:].opt()], outs=[vb_out_a[:].opt()])
            nc.gpsimd.collective_compute(
                "AllGather", mybir.AluOpType.bypass, replica_groups=groups,
                ins=[vb_in_b[:].opt()], outs=[vb_out_b[:].opt()])

            # peer halves: gather the peer's rows of the AllGather outputs
            # straight into the peer tiles — idx data is per-core
            nc.gpsimd.dma_gather(kTp[:, 0:4, :],
                                 kb_out_a[:].rearrange("r d s -> (r d) s"),
                                 gidxk[:, :], S // 4, S // 4, SQ)
            nc.gpsimd.dma_gather(kTp[:, 4:8, :],
                                 kb_out_b[:].rearrange("r d s -> (r d) s"),
                                 gidxk[:, :], S // 4, S // 4, SQ)
            nc.gpsimd.dma_gather(vp[:, 0:4, :],
                                 vb_out_a[:].rearrange("r s d -> (r s) d"),
                                 gidxk[:, :], S // 4, S // 4, D)
            nc.gpsimd.dma_gather(vp[:, 4:8, :],
                                 vb_out_b[:].rearrange("r s d -> (r s) d"),
                                 gidxk[:, :], S // 4, S // 4, D)

            # qT : lhsT = Wq-slice, rhs = xTq
            for te in range(DT):
                for c in range(SQ // N512):
                    ps = psum.tile([P, N512], f32, tag="mm")
                    for u in range(KP):
                        nc.tensor.matmul(ps, wq_sb[:, 2 * u:2 * u + 2, ts(te, P)],
                                         xTq[:, 2 * u:2 * u + 2, ts(c, N512)],
                                         start=(u == 0), stop=(u == KP - 1),
                                         perf_mode=DR)
                    if c == 0:
                        nc.scalar.activation(qT[:, te, ts(c, N512)], ps,
                                             AF.Identity, bias=bqk[:, 0, te:te + 1])
                    else:
                        nc.vector.tensor_scalar_add(qT[:, te, ts(c, N512)], ps,
                                                    bqk[:, 0, te:te + 1])

        # ---------------- phase 2: attention + out-proj ----------------
        with ExitStack() as ph2:
            epool = ph2.enter_context(tc.tile_pool(name="ep", bufs=2))
            work = ph2.enter_context(tc.tile_pool(name="wk2", bufs=2))
            opool = ph2.enter_context(tc.tile_pool(name="op2", bufs=2))
            rspool = ph2.enter_context(tc.tile_pool(name="rs2", bufs=2))
            rpool = ph2.enter_context(tc.tile_pool(name="rp", bufs=8))
            dpool = ph2.enter_context(
                tc.tile_pool(name="dram2", bufs=2, space="DRAM"))

            # scores in own-half / peer-half blocks; each chunk's softmax
            # denominator + reciprocal round-trip hides behind later blocks
            expTs, rSbs = [], []
            for c in range(SQ // N512):          # sq chunks of 512
                expT = epool.tile([P, ST, N512], f8, tag="expT",
                                  name=f"expT{c}")
                expTs.append(expT)

            def sc_block(c, tk_lo, tk_hi):
                expT = expTs[c]
                for tk in range(tk_lo, tk_hi):
                    ksrc = kT if tk < SQT else kTp
                    ps = psum.tile([P, N512], f32, tag="mm")
                    for u in range(KP):
                        nc.tensor.matmul(ps,
                                         ksrc[:, 2 * u:2 * u + 2,
                                              ts(tk % SQT, P)],
                                         qT[:, 2 * u:2 * u + 2, ts(c, N512)],
                                         start=(u == 0), stop=(u == KP - 1),
                                         perf_mode=DR)
                    nc.scalar.activation(expT[:, tk, :], ps, AF.Exp,
                                         scale=EXP_SCALE, bias=ebias)

            def s_block(c):
                expT = expTs[c]
                psS = psum_s.tile([1, N512], f32, tag="S")
                for tk in range(ST // 2):
                    nc.tensor.matmul(psS, ones[:, :, :1],
                                     expT[:, 2 * tk:2 * tk + 2, :],
                                     start=(tk == 0), stop=(tk == ST // 2 - 1),
                                     perf_mode=DR)
                rS_row = rspool.tile([1, N512], f32, tag="rS_row",
                                     name=f"rS{c}")
                nc.vector.reciprocal(rS_row, psS)   # = 32 / Sigma exp
                # broadcast 1/S across partitions via DRAM (stride-0 DMA)
                rs_dram = dpool.tile([N512], f32, tag="rs_dram")
                nc.sync.dma_start(
                    rs_dram[:].rearrange("(o s) -> o s", o=1), rS_row)
                rSb = rspool.tile([P, N512], f32, tag="rSb", name=f"rSb{c}")
                nc.scalar.dma_start(rSb, rs_dram[:].partition_broadcast(P))
                rSbs.append(rSb)

            sc_block(0, 0, SQT)        # own keys: no collective dependency
            sc_block(1, 0, SQT)        # more own-key work to hide the wire
            sc_block(0, SQT, ST)       # peer keys: needs AG1 + gathers
            s_block(0)
            sc_block(1, SQT, ST)
            s_block(1)

            # residual prefetch AFTER the exchange window so its HBM reads
            # don't fight the AllGather wire + gathers (bf16: half traffic)
            res_sbs = []
            for sl8 in range(SQT):
                res_sb = rpool.tile([P, D], bf16, tag="res", name=f"res{sl8}")
                nc.sync.dma_start(res_sb, resid.ap()[sl8 * P:(sl8 + 1) * P, :])
                res_sbs.append(res_sb)

            for c in range(SQ // N512):
                expT, rSb = expTs[c], rSbs[c]
                # mixedUT[d, sq] = (v^T-stationary @ expT) / S  (normalized on
                # the psum->fp8 convert; unnormalized would overflow e4m3)
                mixUT = work.tile([P, DT, N512], f8, tag="mixUT",
                                  name=f"mixUT{c}")
                for dsl in range(DT):
                    ps = psum.tile([P, N512], f32, tag="mm")
                    for tk in range(ST // 2):
                        vsrc = v if tk < SQT // 2 else vp
                        nc.tensor.matmul(ps,
                                         vsrc[:, (2 * tk) % SQT:
                                              (2 * tk) % SQT + 2, ts(dsl, P)],
                                         expT[:, 2 * tk:2 * tk + 2, :],
                                         start=(tk == 0), stop=(tk == ST // 2 - 1),
                                         perf_mode=DR)
                    nc.vector.tensor_mul(mixUT[:, dsl, :], ps, rSb)

                for sl in range(4):
                    row = (c * 4 + sl) * P
                    res_sb = res_sbs[c * 4 + sl]
                    out_sb = opool.tile([P, D], f32, tag="osb")
                    osc = opool.tile([P, N512], f32, tag="osc")
                    for ec in range(D // N512):
                        ps = psum.tile([P, N512], f32, tag="mm")
                        for u in range(KP):
                            nc.tensor.matmul(
                                ps, mixUT[:, 2 * u:2 * u + 2, ts(sl, P)],
                                wo_sb[:, 2 * u:2 * u + 2, ts(ec, N512)],
                                start=(u == 0), stop=(u == KP - 1),
                                perf_mode=DR)
                        # out = psum / 1024 + (residual + bo); alternate the
                        # evict between DVE (fused) and ACT+GpSimd
                        if ec == 0:
                            nc.vector.scalar_tensor_tensor(
                                out_sb[:, ts(ec, N512)], ps, OUT_DESCALE,
                                res_sb[:, ts(ec, N512)], ALU.mult, ALU.add)
                        else:
                            nc.scalar.activation(osc, ps, AF.Identity,
                                                 scale=OUT_DESCALE)
                            nc.gpsimd.tensor_add(out_sb[:, ts(ec, N512)], osc,
                                                 res_sb[:, ts(ec, N512)])
                    nc.sync.dma_start(out_d.ap()[row:row + P, :], out_sb)

    nc.compile()
    return nc


def _get_program():
    if "nc" not in _COMPILED:
        _COMPILED["nc"] = _build_program()
    return _COMPILED["nc"]


def make_in_maps(tokens, Wq, bq, Wk, bk, Wv, bv, Wo, bo):
    tokens = np.asarray(tokens, dtype=np.float32)
    bf = ml_dtypes.bfloat16
    f8 = ml_dtypes.float8_e4m3
    wq_b = np.ascontiguousarray((np.asarray(Wq, np.float32) * WSCALE).astype(f8))
    wk_b = np.ascontiguousarray((np.asarray(Wk, np.float32) * WSCALE).astype(f8))
    wv_b = np.ascontiguousarray((np.asarray(Wv, np.float32) * WSCALE).astype(f8))
    wo_b = np.ascontiguousarray((np.asarray(Wo, np.float32) * WSCALE).astype(f8))
    bq = np.asarray(bq, np.float32) * WSCALE
    bk = np.asarray(bk, np.float32) * WSCALE
    # center v by c ~ E_k[v] so the fp8 mixUT quantizes the small AC part;
    # softmax weights sum to 1, so out = (mixed-c)@Wo + (c@Wo + bo) + resid.
    wv32 = np.asarray(Wv, np.float32)
    cvec = GELU_MEAN * wv32.sum(axis=0) + np.asarray(bv, np.float32)
    bv = (np.asarray(bv, np.float32) - cvec) * WSCALE
    bo_eff = (np.asarray(bo, np.float32)
              + cvec @ np.asarray(Wo, np.float32)).astype(np.float32)

    pp, mm = np.meshgrid(np.arange(P), np.arange(S // 64), indexing="ij")
    base_k = (mm * 16 + (pp % 16)).astype(np.int16)     # j = m*16 + lane

    in_maps = []
    for c in range(NCORES):
        b, h = divmod(c, 2)
        q_rows = tokens[b, h * SQ:(h + 1) * SQ]
        in_maps.append({
            "tokTq": np.ascontiguousarray(q_rows.T.astype(bf)),  # [D, SQ]
            "resid": np.ascontiguousarray((q_rows + bo_eff).astype(bf)),
            "wq": wq_b, "wk": wk_b, "wv": wv_b, "wo": wo_b,
            "bq": bq, "bk": bk, "bv": bv,
            "gidxk": np.ascontiguousarray(base_k + np.int16((1 - h) * (SQ // 2))),
        })
    return in_maps


def gather_out(results):
    out = np.empty((B, S, D), np.float32)
    for c in range(NCORES):
        b, h = divmod(c, 2)
        out[b, h * SQ:(h + 1) * SQ] = results[c]["out"]
    return out


def kernel(tokens, Wq, bq, Wk, bk, Wv, bv, Wo, bo):
    from concourse.bass_utils import run_bass_kernel_spmd

    in_maps = make_in_maps(tokens, Wq, bq, Wk, bk, Wv, bv, Wo, bo)
    nc = _get_program()
    res = run_bass_kernel_spmd(nc, in_maps, core_ids=list(range(NCORES)),
                               trace=False)
    return gather_out(res.results)
